# revision 1
# baseline (speedup 1.0000x reference)
"""BlanchotianAttention TRN2 kernel: 8 NeuronCores, data-parallel over batch (2)
x tensor-parallel over heads (4 heads/core).

Layout strategy (per core, batch b, head-group hg = heads h0..h0+3):
  - host passes xT = x[b].T  [1024, 2048]
  - stage A-qk: qkvT = w.T @ xT -> QT/KT in [d, seq] layout (head-pair tiles)
    q-weights pre-scaled by dim^-0.5 / temperature_h on host.
  - stage A-v: V = x @ w_v in [seq, d] layout, augmented per head with a ones
    block: V_aug[j, h*128 : h*128+128] = [v_h (64) | ones (64)].
  - stage B/C (per 512-wide i-chunk, per head pair, per 128-wide j-tile):
    S^T = K @ Q^T via row-packed matmuls (2 heads in PE rows 0-63 / 64-127),
    P = exp(S^T) on ACT (one [128,1024] activation covers both heads),
    PV+l fused: matmul(lhsT=[v_h | ones], rhs=P) accumulates attn@v in PSUM
    rows 0-63 and the softmax denominator (broadcast) in rows 64-127.
  - void token: the void QUERY's output row is dropped by the reference, so it
    is never computed. The void KEY/VALUE occupy j=2048 inside j-tile 16,
    zero-padded to 128 rows; a per-partition exp bias of -100 on that tile
    zeroes the pad rows' contributions.
  - normalize: reciprocal of the l rows + cross-base multiply -> O^T pair tile.
  - stage D: y_partial = O_norm @ w_out_shard; host sums partials over the 4
    head-group cores of each batch (+ b_out).

All matmul operands are float32r (full-rate fp32, ~1.2e-4 rounding).

Scheduling notes (cost-model driven):
  - ACT (exp) is the steady-state bottleneck; the jt loop is software-pipelined
    (scores emitted one j-tile ahead) and stage A is interleaved with the first
    i-chunk quarter-by-quarter so ACT starts as early as possible.
  - PSUM slot tags are all explicit per-pair/per-head because the Tile slot
    allocator reuses the most-recently-freed slot (LIFO), which otherwise
    chains consumers onto the newest producer and serializes PE<->ACT.
"""
import sys

sys.path.insert(0, "/opt/trn_rl_repo")

import numpy as np

DIM, HEADS, B, N = 1024, 16, 2, 2048
D = DIM // HEADS          # 64
HPC = HEADS // 4          # heads per core = 4
NJT = 17                  # j tiles (16 full + void/pad tile)
P = 128

_cache = {}


def _build():
    import concourse.bass as bass
    import concourse.mybir as mybir
    import concourse.tile as tile
    from concourse import bacc

    F32 = mybir.dt.float32
    F32R = mybir.dt.float32r
    Exp = mybir.ActivationFunctionType.Exp

    nc = bacc.Bacc("TRN2", target_bir_lowering=False, debug=False)
    xT = nc.dram_tensor("xT", [DIM, N], F32, kind="ExternalInput").ap()
    wqkv = nc.dram_tensor("wqkv", [DIM, 768], F32, kind="ExternalInput").ap()
    wout = nc.dram_tensor("wout", [256, DIM], F32, kind="ExternalInput").ap()
    voidk = nc.dram_tensor("voidk", [2, P], F32, kind="ExternalInput").ap()
    voidv = nc.dram_tensor("voidv", [1, 256], F32, kind="ExternalInput").ap()
    ebias_in = nc.dram_tensor("ebias_in", [P, 1], F32, kind="ExternalInput").ap()
    y = nc.dram_tensor("y", [N, DIM], F32, kind="ExternalOutput").ap()

    KO = DIM // P  # 8 k-tiles

    with tile.TileContext(nc) as tc:
        with tc.tile_pool(name="persist", bufs=1) as pp, \
             tc.tile_pool(name="work", bufs=1) as wp, \
             tc.tile_pool(name="psum", bufs=1, space="PSUM") as ps, \
             tc.tile_pool(name="loadA", bufs=2) as lp:

            # ---- constants ----
            ones = pp.tile([P, D], F32)
            nc.vector.memset(ones[:], 1.0)
            ebias = pp.tile([P, 1], F32)
            nc.sync.dma_start(ebias[:], ebias_in)

            # ---- persistent SBUF tensors ----
            qt = pp.tile([P, 2, N], F32R)              # QT head pairs
            kt = pp.tile([P, 2, NJT * P], F32R)        # KT head pairs (+void+pad)
            va = pp.tile([P, NJT, 512], F32R)          # V_aug per j-tile
            wqkv_r = pp.tile([P, KO, 768], F32R)
            wout_r = pp.tile([P, 2, DIM], F32R)
            xT_r = pp.tile([P, KO, N], F32R)

            # ---- DMA + rounding (order = arrival priority) ----
            # interleave wqk/xT-half0 per ko so A-qk(sc0) accumulation can
            # start as soon as the first k-tile lands
            for ko in range(KO):
                stg = lp.tile([P, 1024], F32, tag="stg")
                nc.gpsimd.dma_start(stg[:, 0:768], wqkv[ko * P:(ko + 1) * P, :])
                nc.vector.tensor_copy(wqkv_r[:, ko, :], stg[:, 0:768])
                stg = lp.tile([P, 1024], F32, tag="stg")
                nc.sync.dma_start(stg[:], xT[ko * P:(ko + 1) * P, 0:1024])
                nc.vector.tensor_copy(xT_r[:, ko, 0:1024], stg[:])
            for ko in range(KO):
                stg = lp.tile([P, 1024], F32, tag="stg")
                nc.sync.dma_start(
                    stg[:], xT[ko * P:(ko + 1) * P, 1024:2048])
                nc.vector.tensor_copy(
                    xT_r[:, ko, 1024:2048], stg[:])

            def emit_late_setup():
                # void k columns + pad zeros; V_aug ones blocks + void row;
                # wout load. Deferred past the sc0 prefix so the DVE stream
                # evacuates the first scores' inputs sooner.
                vkt = lp.tile([P, 2], F32, tag="stg")
                nc.sync.dma_start(vkt[:], voidk.rearrange("a p -> p a"))
                for pair in range(2):
                    nc.vector.tensor_copy(kt[:, pair, 2048:2049],
                                          vkt[:, pair:pair + 1])
                    nc.vector.memset(kt[:, pair, 2049:NJT * P].bitcast(F32), 0.0)
                vvt = lp.tile([1, 256], F32, tag="stg")
                nc.sync.dma_start(vvt[:], voidv)
                va16 = va[:, 16, :]
                nc.vector.memset(va16.bitcast(F32), 0.0)
                nc.vector.tensor_copy(
                    va16.rearrange("p (h c) -> p h c", c=P)[0:1, :, 0:D],
                    vvt[:].rearrange("p (h c) -> p h c", c=D))
                for jt in range(NJT):
                    nc.vector.tensor_copy(
                        va[:, jt, :].rearrange("p (h c) -> p h c", c=P)[:, :, D:P],
                        ones[:, None, :].to_broadcast([P, 4, D]))
                for half in range(2):
                    stg = lp.tile([P, 1024], F32, tag="stg")
                    nc.sync.dma_start(stg[:], wout[half * P:(half + 1) * P, :])
                    nc.vector.tensor_copy(wout_r[:, half, :], stg[:])

            # ---- stage A emit helpers ----
            def emit_aqk_ft(sc, ft):
                acc = ps.tile([P, 1024], F32, tag=f"srot{ft % 2}",
                              name=f"aqk_{sc}_{ft}")
                for ko in range(KO):
                    nc.tensor.matmul(
                        acc[:, 0:512],
                        wqkv_r[:, ko, ft * P:(ft + 1) * P],
                        xT_r[:, ko, sc * 512:(sc + 1) * 512],
                        start=(ko == 0), stop=(ko == KO - 1),
                    )
                if ft < 2:
                    nc.vector.tensor_copy(
                        qt[:, ft, sc * 512:(sc + 1) * 512], acc[:, 0:512])
                else:
                    nc.vector.tensor_copy(
                        kt[:, ft - 2, sc * 512:(sc + 1) * 512], acc[:, 0:512])

            def emit_aqk(sc):
                for ft in range(4):  # 0,1: q pairs; 2,3: k pairs
                    emit_aqk_ft(sc, ft)

            def emit_av(st):
                acc = ps.tile([P, 1024], F32, tag=f"srot{st % 2}",
                              name=f"av_{st}")
                for ko in range(KO):
                    nc.tensor.matmul(
                        acc[:, 0:256],
                        xT_r[:, ko, st * P:(st + 1) * P],
                        wqkv_r[:, ko, 512:768],
                        start=(ko == 0), stop=(ko == KO - 1),
                    )
                nc.vector.tensor_copy(
                    va[:, st, :].rearrange("p (h c) -> p h c", c=P)[:, :, 0:D],
                    acc[:, 0:256].rearrange("p (h c) -> p h c", c=D))

            # ---- stage B/C/D emit helpers ----
            def emit_scores_pair(ic, jt, pair):
                isl = slice(ic * 512, (ic + 1) * 512)
                jsl = slice(jt * P, (jt + 1) * P)
                s_pair = ps.tile([P, 1024], F32, tag=f"srot{pair}",
                                 name=f"s_{ic}_{jt}_{pair}")
                nc.tensor.matmul(
                    s_pair[:, 0:512],
                    kt[0:D, pair, jsl], qt[0:D, pair, isl],
                    start=True, stop=True)
                nc.tensor.matmul(
                    s_pair[:, 512:1024],
                    kt[D:P, pair, jsl], qt[D:P, pair, isl],
                    start=True, stop=True)
                return s_pair

            def emit_scores(ic, jt):
                isl = slice(ic * 512, (ic + 1) * 512)
                jsl = slice(jt * P, (jt + 1) * P)
                tiles = []
                for pair in range(2):
                    s_pair = ps.tile([P, 1024], F32, tag=f"srot{pair}",
                                     name=f"s_{ic}_{jt}_{pair}")
                    nc.tensor.matmul(
                        s_pair[:, 0:512],
                        kt[0:D, pair, jsl], qt[0:D, pair, isl],
                        start=True, stop=True)
                    nc.tensor.matmul(
                        s_pair[:, 512:1024],
                        kt[D:P, pair, jsl], qt[D:P, pair, isl],
                        start=True, stop=True)
                    tiles.append(s_pair)
                return tiles

            def emit_exp_pvl(ic, jt, s_cur, pvl, nxt, mid=None):
                """exp(jt) ; scores(nxt) ; [mid()] ; pvl(jt)."""
                p_tiles = []
                for pair in range(2):
                    p_pair = wp.tile([P, 1024], F32R, tag=f"pexp{pair}",
                                     bufs=3 if pair == 0 else 2,
                                     name=f"p_{ic}_{jt}_{pair}")
                    if jt == 16:
                        nc.scalar.activation(p_pair[:], s_cur[pair][:], Exp,
                                             bias=ebias[:])
                    else:
                        nc.scalar.activation(p_pair[:], s_cur[pair][:], Exp)
                    p_tiles.append(p_pair)
                s_nxt = emit_scores(*nxt) if nxt is not None else None
                if mid is not None:
                    mid()
                for pair in range(2):
                    for hh in range(2):
                        h = 2 * pair + hh
                        nc.tensor.matmul(
                            pvl[h][:],
                            va[:, jt, h * P:(h + 1) * P],
                            p_tiles[pair][:, hh * 512:(hh + 1) * 512],
                            start=(jt == 0), stop=(jt == 16),
                        )
                return s_nxt

            def emit_norm(ic, pvl):
                """normalize + pre-allocate y psum tiles; returns (osb, yps)."""
                osb = [wp.tile([P, 512], F32R, tag=f"osb{pair}",
                               bufs=2, name=f"osb{pair}_{ic}")
                       for pair in range(2)]
                for h in range(4):
                    pair, hh = divmod(h, 2)
                    r_sb = lp.tile([P, 1024], F32, tag="stg",
                                   name=f"rsb_{ic}_{h}")[:, 0:512]
                    nc.vector.reciprocal(r_sb[D:P, :], pvl[h][D:P, :])
                    nc.vector.tensor_tensor(
                        osb[pair][hh * D:(hh + 1) * D, :],
                        pvl[h][0:D, :], r_sb[D:P, :],
                        mybir.AluOpType.mult)
                yps = [ps.tile([P, 512], F32, tag=f"pvl{k % 4}",
                               name=f"y_{ic}_{k}") for k in range(8)]
                return osb, yps

            def emit_outproj(ic, osb, yps, its=range(4), split_q=False):
                for it in its:
                    ysb = wp.tile([P, DIM], F32, tag="ysb", bufs=2,
                                  name=f"ysb_{ic}_{it}")
                    for oc in range(2):
                        yp = yps[it * 2 + oc]
                        for pair in range(2):
                            nc.tensor.matmul(
                                yp[:],
                                osb[pair][:, it * P:(it + 1) * P],
                                wout_r[:, pair, oc * 512:(oc + 1) * 512],
                                start=(pair == 0), stop=(pair == 1),
                            )
                        nc.vector.tensor_copy(ysb[:, oc * 512:(oc + 1) * 512],
                                              yp[:])
                    eng = nc.gpsimd if (split_q and it % 2) else nc.sync
                    eng.dma_start(
                        y[ic * 512 + it * P: ic * 512 + (it + 1) * P, :], ysb[:])

            def alloc_pvl(ic):
                return [ps.tile([P, 512], F32, tag=f"pvl{h}", name=f"pvl{h}_{ic}")
                        for h in range(4)]

            # ---- main schedule ----
            # ic0 interleaved with stage A quarter-by-quarter; ic1..3 plain.
            pvl = alloc_pvl(0)
            emit_aqk_ft(0, 0)
            emit_aqk_ft(0, 2)
            s00_p0 = emit_scores_pair(0, 0, 0)
            emit_aqk_ft(0, 1)
            emit_aqk_ft(0, 3)
            s00_p1 = emit_scores_pair(0, 0, 1)
            emit_late_setup()
            for st in range(0, 4):
                emit_av(st)
            s_cur = [s00_p0, s00_p1]
            for jt in range(0, 3):
                s_cur = emit_exp_pvl(0, jt, s_cur, pvl, (0, jt + 1))
            for q in range(1, 4):
                emit_aqk_ft(q, 2)
                emit_aqk_ft(q, 3)
                for st in range(4 * q, 4 * q + 4):
                    emit_av(st)
                lo, hi = 4 * q - 1, 4 * q + 3   # jts whose next-scores live in sc q
                for jt in range(lo, hi if q < 3 else NJT):
                    nxt = (0, jt + 1) if jt < NJT - 1 else (1, 0)
                    s_cur = emit_exp_pvl(0, jt, s_cur, pvl, nxt)
                if q == 1:
                    # sc1 q-features feed ic1's scores (start at ic0-jt16)
                    emit_aqk_ft(q, 0)
                    emit_aqk_ft(q, 1)

            pvl_prev = pvl
            for ic in range(1, 4):
                osb, yps = emit_norm(ic - 1, pvl_prev)
                pvl = alloc_pvl(ic)
                for jt in range(NJT):
                    if jt == NJT - 1:
                        nxt = (ic + 1, 0) if ic < 3 else None
                    else:
                        nxt = (ic, jt + 1)
                    s_cur = emit_exp_pvl(ic, jt, s_cur, pvl, nxt)
                    if jt == 0:
                        emit_outproj(ic - 1, osb, yps)
                    if jt == 2 and ic < 3:
                        # sc(ic+1) q-features, needed by ic(ic+1)'s scores
                        emit_aqk_ft(ic + 1, 0)
                        emit_aqk_ft(ic + 1, 1)
                pvl_prev = pvl
            osb, yps = emit_norm(3, pvl_prev)
            emit_outproj(3, osb, yps)

    nc.compile()
    return nc


def _prep_inputs(x, w_qkv, w_out, b_out, void_q, void_k, void_v,
                 attention_trace, temperature_factor):
    """Host-side sharding / layout prep. Returns in_maps for 8 cores."""
    temp = np.maximum(1.0 + np.abs(attention_trace) * temperature_factor,
                      1.0).reshape(HEADS).astype(np.float32)
    scale = (DIM ** -0.5) / temp                       # [16] per head
    qcol_scale = np.repeat(scale, D)                   # [1024]
    wq_scaled = (w_qkv[:, 0:DIM] * qcol_scale[None, :]).astype(np.float32)
    wk = w_qkv[:, DIM:2 * DIM]
    wv_full = w_qkv[:, 2 * DIM:3 * DIM]
    vk = void_k.reshape(HEADS, D)
    vv = void_v.reshape(HEADS, D)

    ebias = np.zeros((P, 1), np.float32)
    ebias[1:, 0] = -100.0

    in_maps = []
    for core in range(8):
        b, hg = divmod(core, 4)
        h0 = hg * HPC
        cs = slice(h0 * D, (h0 + HPC) * D)             # 256 feature cols
        in_maps.append({
            "xT": np.ascontiguousarray(x[b].T),
            "wqkv": np.ascontiguousarray(
                np.concatenate([wq_scaled[:, cs], wk[:, cs],
                                wv_full[:, cs]], axis=1)),
            "wout": np.ascontiguousarray(w_out[cs, :]),
            "voidk": np.ascontiguousarray(vk[h0:h0 + HPC].reshape(2, P)),
            "voidv": np.ascontiguousarray(vv[h0:h0 + HPC].reshape(1, 256)),
            "ebias_in": ebias,
        })
    return in_maps


def _run(in_maps, trace=False):
    from concourse import bass_utils
    if "nc" not in _cache:
        _cache["nc"] = _build()
    return bass_utils.run_bass_kernel_spmd(
        _cache["nc"], in_maps, core_ids=list(range(8)), trace=trace)


def kernel(x, w_qkv, w_out, b_out, void_q, void_k, void_v,
           attention_trace, temperature_factor):
    args = [np.asarray(a, dtype=np.float32) for a in
            (x, w_qkv, w_out, b_out, void_q, void_k, void_v,
             attention_trace, temperature_factor)]
    in_maps = _prep_inputs(*args)
    res = _run(in_maps)
    out = np.zeros((B, N, DIM), np.float32)
    for core in range(8):
        b = core // 4
        out[b] += res.results[core]["y"]
    out += args[3][None, None, :]                      # b_out
    return out



# revision 18
# speedup vs baseline: 1.2497x; 1.2497x over previous
"""BlanchotianAttention TRN2 kernel: 8 NeuronCores, data-parallel over batch (2)
x tensor-parallel over heads (4 heads/core).

v2 design (pair-phased schedule):
  - Host ships xT/wqkv/wout as bf16; DMA lands directly in the matmul operand
    tiles (no fp32 staging or rounding copies). Whole-tensor DMAs via
    "(ko p) c -> p ko c" rearranges: one descriptor batch per issue.
  - Stage A (qkv projection) in bf16; outputs copied PSUM->SBUF as f32r
    (qt/kt/va layouts as in v1: QT/KT in [d, seq] head-pair tiles, V_aug
    [seq, v|ones] per j-tile, q pre-scaled by dim^-0.5/temperature on host).
  - Main loop is 8 phases = (i-chunk 0..3) x (head pair 0..1); each phase
    sweeps 17 j-tiles. Per (phase, jt): 2 score matmuls -> one [128,1024]
    exp on ACT -> 2 PV matmuls accumulating [v|ones] into the phase's pvl
    bank-set. PSUM: 2 alternating score tiles (2 banks each) + 2 pvl
    bank-sets (2 banks each) = 8 banks; the set idle in a phase is borrowed
    by stage-A accumulators and outproj tiles.
  - Normalize = single tensor_tensor divide per head (DVE + Pool split).
  - Out projection in bf16 (osb bf16 x wout bf16), ysb copies on Pool,
    y partials DMA'd as bf16; host sums partials in fp32 and adds b_out.
  - Void token: j-tile 16 holds the void key (zero-padded); per-partition
    exp bias of -100 kills the pad rows. Void QUERY row is never computed.
"""
import sys

sys.path.insert(0, "/opt/trn_rl_repo")

import numpy as np

DIM, HEADS, B, N = 1024, 16, 2, 2048
D = DIM // HEADS          # 64
HPC = HEADS // 4          # heads per core = 4
NJT = 17                  # j tiles (16 full + void/pad tile)
P = 128
KO = DIM // P             # 8 k-tiles

_cache = {}


def _build():
    import concourse.bass as bass
    import concourse.mybir as mybir
    import concourse.tile as tile
    from concourse import bacc

    F32 = mybir.dt.float32
    F32R = mybir.dt.float32r
    BF16 = mybir.dt.bfloat16
    Exp = mybir.ActivationFunctionType.Exp
    Div = mybir.AluOpType.divide

    nc = bacc.Bacc("TRN2", target_bir_lowering=False, debug=False)
    xT = nc.dram_tensor("xT", [DIM, N], BF16, kind="ExternalInput").ap()
    wqkv = nc.dram_tensor("wqkv", [DIM, 768], BF16, kind="ExternalInput").ap()
    wout = nc.dram_tensor("wout", [256, DIM], BF16, kind="ExternalInput").ap()
    voidk = nc.dram_tensor("voidk", [2, P], F32, kind="ExternalInput").ap()
    voidvo = nc.dram_tensor("voidvo", [2, 2, P], F32R,
                            kind="ExternalInput").ap()
    y = nc.dram_tensor("y", [N, DIM], BF16, kind="ExternalOutput").ap()

    xT_r = xT.rearrange("(ko p) s -> p ko s", p=P)
    wqkv_r = wqkv.rearrange("(ko p) c -> p ko c", p=P)
    wout_r = wout.rearrange("(k p) c -> p k c", p=P)

    with tile.TileContext(nc) as tc:
        with tc.tile_pool(name="persist", bufs=1) as pp, \
             tc.tile_pool(name="work", bufs=1) as wp, \
             tc.tile_pool(name="psum", bufs=1, space="PSUM") as ps:

            # ---- persistent SBUF ----
            xT_bf = pp.tile([P, KO, N], BF16)
            wqkv_bf = pp.tile([P, KO, 768], BF16)
            wout_bf = pp.tile([P, 2, DIM], BF16)
            qt = pp.tile([P, 2, N], F32R)
            kt = pp.tile([P, 2, 2048], F32R)
            va = pp.tile([P, 16, 512], F32R)
            ones = pp.tile([P, D], F32)
            vkt = pp.tile([P, 2], F32)
            kvbd = pp.tile([P, 2, P], F32R)    # block-diag void-key lhsT
            vones = pp.tile([P, 2, P], F32R)   # [v_h | ones] rank-1 lhsT

            # ---- DMA issues (all SP queue; priority order) ----
            nc.sync.dma_start(wqkv_bf[:, :, 0:256], wqkv_r[:, :, 0:256])
            nc.sync.dma_start(xT_bf[:, :, 0:512], xT_r[:, :, 0:512])
            nc.sync.dma_start(wqkv_bf[:, :, 256:512], wqkv_r[:, :, 256:512])
            nc.sync.dma_start(wqkv_bf[:, :, 512:768], wqkv_r[:, :, 512:768])
            nc.sync.dma_start(xT_bf[:, :, 512:1024], xT_r[:, :, 512:1024])
            nc.sync.dma_start(xT_bf[:, :, 1024:1536], xT_r[:, :, 1024:1536])
            nc.sync.dma_start(xT_bf[:, :, 1536:2048], xT_r[:, :, 1536:2048])
            nc.sync.dma_start(wout_bf[:], wout_r)
            nc.sync.dma_start(vkt[:], voidk.rearrange("a p -> p a"))
            nc.sync.dma_start(vones[0:1, :, :], voidvo[0:1, :, :])
            nc.sync.dma_start(vones[64:65, :, :], voidvo[1:2, :, :])

            # ---- setup on Pool (keeps DVE free for stage-A copies) ----
            nc.gpsimd.memset(ones[:], 1.0)
            nc.gpsimd.memset(kvbd[:].bitcast(F32), 0.0)
            for pair in range(2):
                for hh in range(2):
                    hs = slice(hh * D, (hh + 1) * D)
                    nc.gpsimd.tensor_copy(
                        kvbd[hs, pair, hs],
                        vkt[hs, pair:pair + 1].to_broadcast([D, D]))
            for jt in range(16):
                nc.gpsimd.tensor_copy(
                    va[:, jt, :].rearrange("p (h c) -> p h c", c=P)[:, :, D:P],
                    ones[:, None, :].to_broadcast([P, 4, D]))

            # ---- PE pipeline warmup: zero-data matmuls keep the tensor
            # engine busy (and its p-state ramping) while input DMA lands ----
            wsrc = pp.tile([P, 512], BF16)
            nc.vector.memset(wsrc[:].bitcast(mybir.dt.uint16), 0)

            # ---- stage A (borrows the idle pvl bank-set) ----
            st8 = {"other": 1, "slot": 0}

            def btag():
                t = f"pvl{st8['other']}{st8['slot']}"
                st8["slot"] ^= 1
                return t

            def emit_aqk(sc, ft):
                acc = ps.tile([P, 512], F32, tag=btag(), name=f"aqk_{sc}_{ft}")
                for ko in range(KO):
                    nc.tensor.matmul(
                        acc[:],
                        wqkv_bf[:, ko, ft * P:(ft + 1) * P],
                        xT_bf[:, ko, sc * 512:(sc + 1) * 512],
                        start=(ko == 0), stop=(ko == KO - 1),
                    )
                if ft < 2:
                    nc.vector.tensor_copy(qt[:, ft, sc * 512:(sc + 1) * 512],
                                          acc[:])
                else:
                    nc.vector.tensor_copy(kt[:, ft - 2, sc * 512:(sc + 1) * 512],
                                          acc[:])

            def emit_av(st):
                acc = ps.tile([P, 512], F32, tag=btag(), name=f"av_{st}")
                for ko in range(KO):
                    nc.tensor.matmul(
                        acc[:, 0:256],
                        xT_bf[:, ko, st * P:(st + 1) * P],
                        wqkv_bf[:, ko, 512:768],
                        start=(ko == 0), stop=(ko == KO - 1),
                    )
                nc.vector.tensor_copy(
                    va[:, st, :].rearrange("p (h c) -> p h c", c=P)[:, :, 0:D],
                    acc[:, 0:256].rearrange("p (h c) -> p h c", c=D))

            # ---- main loop pieces ----
            sidx = {"i": 0}

            def emit_scores(ic, pair, jt):
                isl = slice(ic * 512, (ic + 1) * 512)
                jsl = slice(jt * P, (jt + 1) * P)
                i = sidx["i"]
                sidx["i"] += 1
                s = ps.tile([P, 1024], F32, tag=f"s{i % 2}",
                            name=f"s_{ic}_{pair}_{jt}")
                for hh in range(2):
                    nc.tensor.matmul(
                        s[:, hh * 512:(hh + 1) * 512],
                        kt[hh * D:(hh + 1) * D, pair, jsl],
                        qt[hh * D:(hh + 1) * D, pair, isl],
                        start=True, stop=True)
                return s

            def emit_exp_pvl(ic, pair, jt, s_cur, pvl, nxt, hook):
                p = wp.tile([P, 1024], F32R, tag="pexp", bufs=3,
                            name=f"p_{ic}_{pair}_{jt}")
                nc.scalar.activation(p[:], s_cur[:], Exp)
                s_nxt = emit_scores(*nxt) if nxt is not None else None
                if hook is not None:
                    hook()
                for hh in range(2):
                    h = 2 * pair + hh
                    nc.tensor.matmul(
                        pvl[hh][:],
                        va[:, jt, h * P:(h + 1) * P],
                        p[:, hh * 512:(hh + 1) * 512],
                        start=(jt == 0), stop=False,
                    )
                return s_nxt

            # void key: one block-diagonal matmul gives both heads' void
            # scores row-replicated ([0:64]=head0, [64:128]=head1), one
            # [128,512] exp, then rank-1 [v|ones] x exp(s_void) closes pvl
            def emit_void_scores(ic, pair):
                isl = slice(ic * 512, (ic + 1) * 512)
                vs = ps.tile([P, 512], F32, tag=btag(), name=f"vs_{ic}_{pair}")
                nc.tensor.matmul(vs[:], kvbd[:, pair, :], qt[:, pair, isl],
                                 start=True, stop=True)
                return vs

            def emit_void_exp(vs, ic, pair):
                vse = wp.tile([P, 512], F32R, tag="vse", bufs=2,
                              name=f"vse_{ic}_{pair}")
                nc.scalar.activation(vse[:], vs[:], Exp)
                return vse

            def emit_void_pvl(pair, pvl, vse):
                for hh in range(2):
                    nc.tensor.matmul(
                        pvl[hh][:],
                        vones[hh * D:hh * D + 1, pair, :],
                        vse[hh * D:hh * D + 1, :],
                        start=False, stop=True)

            def emit_norm(ic, pair, pvl):
                # BIR allows only one PSUM operand per instruction: move one
                # side to SBUF first. head0 via DVE recip+mult, head1 via
                # Pool copy+divide.
                osb = wp.tile([P, 512], BF16, tag=f"osb{pair}", bufs=2,
                              name=f"osb_{ic}_{pair}")
                for hh in range(2):
                    r_sb = wp.tile([D, 512], F32, tag=f"rsb{hh}", bufs=2,
                                   name=f"rsb_{ic}_{pair}_{hh}")
                    nc.vector.reciprocal(r_sb[:], pvl[hh][D:P, :])
                    nc.vector.tensor_tensor(osb[hh * D:(hh + 1) * D, :],
                                            pvl[hh][0:D, :], r_sb[:],
                                            mybir.AluOpType.mult)
                return osb

            def emit_outproj_it(ic, it, osbs):
                ysb = wp.tile([P, DIM], BF16, tag="ysb", bufs=2,
                              name=f"ysb_{ic}_{it}")
                for oc in range(2):
                    yp = ps.tile([P, 512], F32, tag=btag(),
                                 name=f"y_{ic}_{it}_{oc}")
                    for pair in range(2):
                        nc.tensor.matmul(
                            yp[:],
                            osbs[pair][:, it * P:(it + 1) * P],
                            wout_bf[:, pair, oc * 512:(oc + 1) * 512],
                            start=(pair == 0), stop=(pair == 1),
                        )
                    nc.vector.tensor_copy(ysb[:, oc * 512:(oc + 1) * 512],
                                          yp[:])
                nc.sync.dma_start(
                    y[ic * 512 + it * P: ic * 512 + (it + 1) * P, :], ysb[:])

            def emit_outproj_pre(ic, it, osb0):
                yps = []
                for oc in range(2):
                    yp = ps.tile([P, 512], F32, tag=btag(),
                                 name=f"y_{ic}_{it}_{oc}")
                    nc.tensor.matmul(
                        yp[:],
                        osb0[:, it * P:(it + 1) * P],
                        wout_bf[:, 0, oc * 512:(oc + 1) * 512],
                        start=True, stop=False,
                    )
                    yps.append(yp)
                return yps

            def emit_outproj_fin(ic, it, osb1, yps):
                ysb = wp.tile([P, DIM], BF16, tag="ysb", bufs=2,
                              name=f"ysb_{ic}_{it}")
                for oc in range(2):
                    nc.tensor.matmul(
                        yps[oc][:],
                        osb1[:, it * P:(it + 1) * P],
                        wout_bf[:, 1, oc * 512:(oc + 1) * 512],
                        start=False, stop=True,
                    )
                    nc.vector.tensor_copy(ysb[:, oc * 512:(oc + 1) * 512],
                                          yps[oc][:])
                nc.sync.dma_start(
                    y[ic * 512 + it * P: ic * 512 + (it + 1) * P, :], ysb[:])

            # ---- hook schedule ----
            osbs = {}
            yps_pre = {}

            def oj(ic, it):
                def f():
                    emit_outproj_it(ic, it, [osbs[(ic, 0)], osbs[(ic, 1)]])
                return f

            def ojp(ic, it):
                def f():
                    yps_pre[it] = emit_outproj_pre(ic, it, osbs[(ic, 0)])
                return f

            def aqk(sc, ft):
                return lambda: emit_aqk(sc, ft)

            def av(*sts):
                return lambda: [emit_av(st) for st in sts]

            hooks = {
                (0, 0): av(0, 1), (0, 1): av(2, 3), (0, 2): aqk(1, 2),
                (0, 3): av(4, 5), (0, 4): av(6, 7), (0, 5): aqk(2, 2),
                (0, 6): av(8, 9), (0, 7): av(10, 11), (0, 8): aqk(3, 2),
                (0, 9): av(12, 13), (0, 10): av(14, 15), (0, 11): aqk(0, 1),
                (0, 12): aqk(0, 3), (0, 14): aqk(1, 3),
                (1, 2): aqk(2, 3), (1, 5): aqk(3, 3), (1, 8): aqk(1, 0),
                (2, 0): oj(0, 0), (2, 2): oj(0, 1), (2, 4): oj(0, 2),
                (2, 6): oj(0, 3), (2, 8): aqk(1, 1),
                (3, 8): aqk(2, 0),
                (4, 0): oj(1, 0), (4, 2): oj(1, 1), (4, 4): oj(1, 2),
                (4, 6): oj(1, 3), (4, 8): aqk(2, 1),
                (5, 8): aqk(3, 0),
                (6, 0): oj(2, 0), (6, 2): oj(2, 1), (6, 4): oj(2, 2),
                (6, 6): oj(2, 3), (6, 8): aqk(3, 1),
                (7, 9): ojp(3, 0), (7, 11): ojp(3, 1),
                (7, 13): ojp(3, 2), (7, 15): ojp(3, 3),
            }

            # ---- main schedule ----
            phases = [(ic, pair) for ic in range(4) for pair in range(2)]

            NWARM = 14
            for w in range(NWARM):
                wacc = ps.tile([P, 512], F32, tag=btag(), name=f"warm_{w}")
                nc.tensor.matmul(wacc[:], wsrc[:, 0:128], wsrc[:],
                                 start=True, stop=True)

            emit_aqk(0, 0)
            emit_aqk(0, 2)
            s_cur = emit_scores(0, 0, 0)
            for pi, (ic, pair) in enumerate(phases):
                sset = pi % 2
                st8["other"] = 1 - sset
                pvl = [ps.tile([P, 512], F32, tag=f"pvl{sset}{hh}",
                               name=f"pvl_{pi}_{hh}") for hh in range(2)]
                vs_cur = vse_cur = None
                for jt in range(16):
                    if jt == 15:
                        nxt = ((phases[pi + 1][0], phases[pi + 1][1], 0)
                               if pi < 7 else None)
                    else:
                        nxt = (ic, pair, jt + 1)
                    s_cur = emit_exp_pvl(ic, pair, jt, s_cur, pvl, nxt,
                                         hooks.get((pi, jt)))
                    if jt == 1:
                        vs_cur = emit_void_scores(ic, pair)
                    elif jt == 3:
                        vse_cur = emit_void_exp(vs_cur, ic, pair)
                emit_void_pvl(pair, pvl, vse_cur)
                osbs[(ic, pair)] = emit_norm(ic, pair, pvl)
            # tail: finish the last i-chunk's out projection
            for it in range(4):
                emit_outproj_fin(3, it, osbs[(3, 1)], yps_pre[it])

    nc.compile()
    return nc


def _voidvo(vv4):
    """[v_h | ones] rank-1 lhsT rows for the void value: [hh, pair, 128]."""
    out = np.ones((2, 2, P), np.float32)
    for pair in range(2):
        for hh in range(2):
            out[hh, pair, 0:D] = vv4[2 * pair + hh]
    return out


def _prep_inputs(x, w_qkv, w_out, b_out, void_q, void_k, void_v,
                 attention_trace, temperature_factor):
    """Host-side sharding / layout prep. Returns in_maps for 8 cores."""
    import ml_dtypes
    BF = ml_dtypes.bfloat16

    temp = np.maximum(1.0 + np.abs(attention_trace) * temperature_factor,
                      1.0).reshape(HEADS).astype(np.float32)
    scale = (DIM ** -0.5) / temp                       # [16] per head
    qcol_scale = np.repeat(scale, D)                   # [1024]
    wq_scaled = (w_qkv[:, 0:DIM] * qcol_scale[None, :]).astype(np.float32)
    wk = w_qkv[:, DIM:2 * DIM]
    wv_full = w_qkv[:, 2 * DIM:3 * DIM]
    vk = void_k.reshape(HEADS, D)
    vv = void_v.reshape(HEADS, D)

    in_maps = []
    for core in range(8):
        b, hg = divmod(core, 4)
        h0 = hg * HPC
        cs = slice(h0 * D, (h0 + HPC) * D)             # 256 feature cols
        in_maps.append({
            "xT": np.ascontiguousarray(x[b].T).astype(BF),
            "wqkv": np.ascontiguousarray(
                np.concatenate([wq_scaled[:, cs], wk[:, cs],
                                wv_full[:, cs]], axis=1)).astype(BF),
            "wout": np.ascontiguousarray(w_out[cs, :]).astype(BF),
            "voidk": np.ascontiguousarray(vk[h0:h0 + HPC].reshape(2, P)),
            "voidvo": _voidvo(vv[h0:h0 + HPC]),
        })
    return in_maps


def _run(in_maps, trace=False):
    from concourse import bass_utils
    if "nc" not in _cache:
        _cache["nc"] = _build()
    return bass_utils.run_bass_kernel_spmd(
        _cache["nc"], in_maps, core_ids=list(range(8)), trace=trace)


def kernel(x, w_qkv, w_out, b_out, void_q, void_k, void_v,
           attention_trace, temperature_factor):
    args = [np.asarray(a, dtype=np.float32) for a in
            (x, w_qkv, w_out, b_out, void_q, void_k, void_v,
             attention_trace, temperature_factor)]
    in_maps = _prep_inputs(*args)
    res = _run(in_maps)
    out = np.zeros((B, N, DIM), np.float32)
    for core in range(8):
        b = core // 4
        out[b] += np.asarray(res.results[core]["y"], dtype=np.float32)
    out += args[3][None, None, :]                      # b_out
    return out


# revision 28
# speedup vs baseline: 1.3002x; 1.0404x over previous
"""BlanchotianAttention TRN2 kernel: 8 NeuronCores, data-parallel over batch (2)
x tensor-parallel over heads (4 heads/core).

v2 design (pair-phased schedule):
  - Host ships xT/wqkv/wout as bf16; DMA lands directly in the matmul operand
    tiles (no fp32 staging or rounding copies). Whole-tensor DMAs via
    "(ko p) c -> p ko c" rearranges: one descriptor batch per issue.
  - Stage A (qkv projection) in bf16; outputs copied PSUM->SBUF as f32r
    (qt/kt/va layouts as in v1: QT/KT in [d, seq] head-pair tiles, V_aug
    [seq, v|ones] per j-tile, q pre-scaled by dim^-0.5/temperature on host).
  - Main loop is 8 phases = (i-chunk 0..3) x (head pair 0..1); each phase
    sweeps 17 j-tiles. Per (phase, jt): 2 score matmuls -> one [128,1024]
    exp on ACT -> 2 PV matmuls accumulating [v|ones] into the phase's pvl
    bank-set. PSUM: 2 alternating score tiles (2 banks each) + 2 pvl
    bank-sets (2 banks each) = 8 banks; the set idle in a phase is borrowed
    by stage-A accumulators and outproj tiles.
  - Normalize = single tensor_tensor divide per head (DVE + Pool split).
  - Out projection in bf16 (osb bf16 x wout bf16), ysb copies on Pool,
    y partials DMA'd as bf16; host sums partials in fp32 and adds b_out.
  - Void token: j-tile 16 holds the void key (zero-padded); per-partition
    exp bias of -100 kills the pad rows. Void QUERY row is never computed.
"""
import sys

sys.path.insert(0, "/opt/trn_rl_repo")

import numpy as np

DIM, HEADS, B, N = 1024, 16, 2, 2048
D = DIM // HEADS          # 64
HPC = HEADS // 4          # heads per core = 4
NJT = 17                  # j tiles (16 full + void/pad tile)
P = 128
KO = DIM // P             # 8 k-tiles

_cache = {}


def _build():
    import concourse.bass as bass
    import concourse.mybir as mybir
    import concourse.tile as tile
    from concourse import bacc

    F32 = mybir.dt.float32
    F32R = mybir.dt.float32r
    BF16 = mybir.dt.bfloat16
    Exp = mybir.ActivationFunctionType.Exp
    Div = mybir.AluOpType.divide

    nc = bacc.Bacc("TRN2", target_bir_lowering=False, debug=False)
    xT = nc.dram_tensor("xT", [DIM, N], BF16, kind="ExternalInput").ap()
    wqkv = nc.dram_tensor("wqkv", [DIM, 768], BF16, kind="ExternalInput").ap()
    wout = nc.dram_tensor("wout", [256, DIM], BF16, kind="ExternalInput").ap()
    voidk = nc.dram_tensor("voidk", [2, P], F32, kind="ExternalInput").ap()
    voidvo = nc.dram_tensor("voidvo", [2, 2, P], F32R,
                            kind="ExternalInput").ap()
    y = nc.dram_tensor("y", [N, DIM], BF16, kind="ExternalOutput").ap()

    xT_r = xT.rearrange("(ko p) s -> p ko s", p=P)
    wqkv_r = wqkv.rearrange("(ko p) c -> p ko c", p=P)
    wout_r = wout.rearrange("(k p) c -> p k c", p=P)

    with tile.TileContext(nc) as tc:
        with tc.tile_pool(name="persist", bufs=1) as pp, \
             tc.tile_pool(name="work", bufs=1) as wp, \
             tc.tile_pool(name="psum", bufs=1, space="PSUM") as ps:

            # ---- persistent SBUF ----
            xT_bf = pp.tile([P, KO, N], BF16)
            wqkv_bf = pp.tile([P, KO, 768], BF16)
            wout_bf = pp.tile([P, 2, DIM], BF16)
            qt = pp.tile([P, 2, N], F32R)
            kt = pp.tile([P, 2, 2048], F32R)
            va = pp.tile([P, 16, 512], F32R)
            ones = pp.tile([P, D], F32)
            vkt = pp.tile([P, 2], F32)
            kvbd = pp.tile([P, 2, P], F32R)    # block-diag void-key lhsT
            vones = pp.tile([P, 2, P], F32R)   # [v_h | ones] rank-1 lhsT

            # ---- DMA issues (all SP queue; priority order) ----
            nc.sync.dma_start(wqkv_bf[:, :, 0:256], wqkv_r[:, :, 0:256])
            nc.sync.dma_start(xT_bf[:, :, 0:256], xT_r[:, :, 0:256])
            nc.sync.dma_start(wqkv_bf[:, :, 256:512], wqkv_r[:, :, 256:512])
            nc.sync.dma_start(xT_bf[:, :, 256:512], xT_r[:, :, 256:512])
            nc.sync.dma_start(wqkv_bf[:, :, 512:768], wqkv_r[:, :, 512:768])
            nc.sync.dma_start(xT_bf[:, :, 512:1024], xT_r[:, :, 512:1024])
            nc.sync.dma_start(xT_bf[:, :, 1024:1536], xT_r[:, :, 1024:1536])
            nc.sync.dma_start(xT_bf[:, :, 1536:2048], xT_r[:, :, 1536:2048])
            nc.sync.dma_start(wout_bf[:], wout_r)
            nc.sync.dma_start(vkt[:], voidk.rearrange("a p -> p a"))
            nc.sync.dma_start(vones[0:1, :, :], voidvo[0:1, :, :])
            nc.sync.dma_start(vones[64:65, :, :], voidvo[1:2, :, :])

            # ---- setup on Pool (keeps DVE free for stage-A copies) ----
            nc.gpsimd.memset(ones[:], 1.0)
            nc.gpsimd.memset(kvbd[:].bitcast(F32), 0.0)
            for pair in range(2):
                for hh in range(2):
                    hs = slice(hh * D, (hh + 1) * D)
                    nc.gpsimd.tensor_copy(
                        kvbd[hs, pair, hs],
                        vkt[hs, pair:pair + 1].to_broadcast([D, D]))
            for jt in range(16):
                nc.gpsimd.tensor_copy(
                    va[:, jt, :].rearrange("p (h c) -> p h c", c=P)[:, :, D:P],
                    ones[:, None, :].to_broadcast([P, 4, D]))

            # ---- PE pipeline warmup: zero-data matmuls keep the tensor
            # engine busy (and its p-state ramping) while input DMA lands ----
            wsrc = pp.tile([P, 512], BF16)
            nc.vector.memset(wsrc[:].bitcast(mybir.dt.uint16), 0)

            # ---- stage A (borrows the idle pvl bank-set) ----
            st8 = {"other": 1, "slot": 0}

            def btag():
                t = f"pvl{st8['other']}{st8['slot']}"
                st8["slot"] ^= 1
                return t

            aqk_accs = {}

            def emit_aqk_part(sc, ft, part, nparts=4):
                kpp = KO // nparts
                if part == 0:
                    aqk_accs[(sc, ft)] = ps.tile([P, 512], F32, tag=btag(),
                                                 name=f"aqk_{sc}_{ft}")
                acc = aqk_accs[(sc, ft)]
                for ko in range(part * kpp, (part + 1) * kpp):
                    nc.tensor.matmul(
                        acc[:],
                        wqkv_bf[:, ko, ft * P:(ft + 1) * P],
                        xT_bf[:, ko, sc * 512:(sc + 1) * 512],
                        start=(ko == 0), stop=(ko == KO - 1),
                    )
                if part == nparts - 1:
                    if ft < 2:
                        nc.vector.tensor_copy(
                            qt[:, ft, sc * 512:(sc + 1) * 512], acc[:])
                    else:
                        nc.vector.tensor_copy(
                            kt[:, ft - 2, sc * 512:(sc + 1) * 512], acc[:])

            def emit_aqk(sc, ft):
                for part in range(4):
                    emit_aqk_part(sc, ft, part)

            def emit_aqk_halves(sc, ft):
                # startup variant: accumulate each 256-col half separately so
                # compute starts as soon as the first half-chunk of xT lands
                acc = ps.tile([P, 512], F32, tag=btag(), name=f"aqkh_{sc}_{ft}")
                for half in range(2):
                    cs = slice(sc * 512 + half * 256, sc * 512 + half * 256 + 256)
                    for ko in range(KO):
                        nc.tensor.matmul(
                            acc[:, half * 256:(half + 1) * 256],
                            wqkv_bf[:, ko, ft * P:(ft + 1) * P],
                            xT_bf[:, ko, cs],
                            start=(ko == 0), stop=(ko == KO - 1),
                        )
                if ft < 2:
                    nc.vector.tensor_copy(qt[:, ft, sc * 512:(sc + 1) * 512],
                                          acc[:])
                else:
                    nc.vector.tensor_copy(kt[:, ft - 2, sc * 512:(sc + 1) * 512],
                                          acc[:])

            def emit_av(st):
                acc = ps.tile([P, 512], F32, tag=btag(), name=f"av_{st}")
                for ko in range(KO):
                    nc.tensor.matmul(
                        acc[:, 0:256],
                        xT_bf[:, ko, st * P:(st + 1) * P],
                        wqkv_bf[:, ko, 512:768],
                        start=(ko == 0), stop=(ko == KO - 1),
                    )
                nc.vector.tensor_copy(
                    va[:, st, :].rearrange("p (h c) -> p h c", c=P)[:, :, 0:D],
                    acc[:, 0:256].rearrange("p (h c) -> p h c", c=D))

            # ---- main loop pieces ----
            sidx = {"i": 0}

            def emit_scores(ic, pair, jt):
                isl = slice(ic * 512, (ic + 1) * 512)
                jsl = slice(jt * P, (jt + 1) * P)
                i = sidx["i"]
                sidx["i"] += 1
                s = ps.tile([P, 1024], F32, tag=f"s{i % 2}",
                            name=f"s_{ic}_{pair}_{jt}")
                for hh in range(2):
                    nc.tensor.matmul(
                        s[:, hh * 512:(hh + 1) * 512],
                        kt[hh * D:(hh + 1) * D, pair, jsl],
                        qt[hh * D:(hh + 1) * D, pair, isl],
                        start=True, stop=True)
                return s

            def emit_exp_pvl(ic, pair, jt, s_cur, pvl, nxt, hook,
                             pvstop=False):
                p = wp.tile([P, 1024], F32R, tag="pexp", bufs=6,
                            name=f"p_{ic}_{pair}_{jt}")
                nc.scalar.activation(p[:], s_cur[:], Exp)
                s_nxt = emit_scores(*nxt) if nxt is not None else None
                if hook is not None:
                    hook()
                for hh in range(2):
                    h = 2 * pair + hh
                    nc.tensor.matmul(
                        pvl[hh][:],
                        va[:, jt, h * P:(h + 1) * P],
                        p[:, hh * 512:(hh + 1) * 512],
                        start=(jt == 0), stop=(jt == 15 and pvstop),
                    )
                return s_nxt

            # void key: one block-diagonal matmul gives both heads' void
            # scores row-replicated ([0:64]=head0, [64:128]=head1), one
            # [128,512] exp, then rank-1 [v|ones] x exp(s_void) closes pvl
            def emit_void_scores(ic, pair):
                isl = slice(ic * 512, (ic + 1) * 512)
                vs = ps.tile([P, 512], F32, tag=btag(), name=f"vs_{ic}_{pair}")
                nc.tensor.matmul(vs[:], kvbd[:, pair, :], qt[:, pair, isl],
                                 start=True, stop=True)
                return vs

            def emit_void_exp(vs, ic, pair):
                vse = wp.tile([P, 512], F32R, tag="vse", bufs=2,
                              name=f"vse_{ic}_{pair}")
                nc.scalar.activation(vse[:], vs[:], Exp)
                return vse

            def emit_void_pvl(pair, pvl, vse, stop):
                for hh in range(2):
                    nc.tensor.matmul(
                        pvl[hh][:],
                        vones[hh * D:hh * D + 1, pair, :],
                        vse[hh * D:hh * D + 1, :],
                        start=False, stop=stop)

            def emit_norm_chunk(ic, pair, pvl, osb, chunk):
                cs = slice(chunk * P, (chunk + 1) * P)
                for hh in range(2):
                    r_sb = wp.tile([D, P], F32, tag=f"rsbs{hh}", bufs=2,
                                   name=f"rsbs_{ic}_{pair}_{hh}_{chunk}")
                    nc.vector.reciprocal(r_sb[:], pvl[hh][D:P, cs])
                    nc.vector.tensor_tensor(
                        osb[hh * D:(hh + 1) * D, cs],
                        pvl[hh][0:D, cs], r_sb[:],
                        mybir.AluOpType.mult)

            def emit_norm(ic, pair, pvl):
                # BIR allows only one PSUM operand per instruction: move one
                # side to SBUF first. head0 via DVE recip+mult, head1 via
                # Pool copy+divide.
                osb = wp.tile([P, 512], BF16, tag=f"osb{pair}", bufs=2,
                              name=f"osb_{ic}_{pair}")
                for hh in range(2):
                    r_sb = wp.tile([D, 512], F32, tag=f"rsb{hh}", bufs=2,
                                   name=f"rsb_{ic}_{pair}_{hh}")
                    nc.vector.reciprocal(r_sb[:], pvl[hh][D:P, :])
                    nc.vector.tensor_tensor(osb[hh * D:(hh + 1) * D, :],
                                            pvl[hh][0:D, :], r_sb[:],
                                            mybir.AluOpType.mult)
                return osb

            def emit_outproj_oc(ic, it, oc, osbs):
                yp = ps.tile([P, 512], F32, tag=btag(),
                             name=f"y_{ic}_{it}_{oc}")
                for pair in range(2):
                    nc.tensor.matmul(
                        yp[:],
                        osbs[pair][:, it * P:(it + 1) * P],
                        wout_bf[:, pair, oc * 512:(oc + 1) * 512],
                        start=(pair == 0), stop=(pair == 1),
                    )
                ysbh = wp.tile([P, 512], BF16, tag="ysbh", bufs=4,
                               name=f"ysbh_{ic}_{it}_{oc}")
                nc.vector.tensor_copy(ysbh[:], yp[:])
                nc.sync.dma_start(
                    y[ic * 512 + it * P: ic * 512 + (it + 1) * P,
                      oc * 512:(oc + 1) * 512], ysbh[:])

            def emit_outproj_pre(ic, it, osb0):
                yps = []
                for oc in range(2):
                    yp = ps.tile([P, 512], F32, tag=btag(),
                                 name=f"y_{ic}_{it}_{oc}")
                    nc.tensor.matmul(
                        yp[:],
                        osb0[:, it * P:(it + 1) * P],
                        wout_bf[:, 0, oc * 512:(oc + 1) * 512],
                        start=True, stop=False,
                    )
                    yps.append(yp)
                return yps

            def emit_outproj_fin(ic, it, osb1, yps):
                for oc in range(2):
                    nc.tensor.matmul(
                        yps[oc][:],
                        osb1[:, it * P:(it + 1) * P],
                        wout_bf[:, 1, oc * 512:(oc + 1) * 512],
                        start=False, stop=True,
                    )
                    ysbh = wp.tile([P, 512], BF16, tag="ysbh", bufs=4,
                                   name=f"ysbh_{ic}_{it}_{oc}")
                    if oc == 0:
                        nc.vector.tensor_copy(ysbh[:], yps[oc][:])
                    else:
                        nc.scalar.copy(ysbh[:], yps[oc][:])
                    nc.sync.dma_start(
                        y[ic * 512 + it * P: ic * 512 + (it + 1) * P,
                          oc * 512:(oc + 1) * 512], ysbh[:])

            # ---- hook schedule ----
            osbs = {}
            yps_pre = {}

            def oj(ic, k):
                def f():
                    emit_outproj_oc(ic, k // 2, k % 2,
                                    [osbs[(ic, 0)], osbs[(ic, 1)]])
                return f

            def ojp(ic, it):
                def f():
                    yps_pre[it] = emit_outproj_pre(ic, it, osbs[(ic, 0)])
                return f

            def aqk(sc, ft, part=None):
                if part is None:
                    return lambda: emit_aqk(sc, ft)
                return lambda: emit_aqk_part(sc, ft, part)

            def av(*sts):
                return lambda: [emit_av(st) for st in sts]

            hooks = {
                (0, 0): av(0, 1), (0, 1): av(2, 3), (0, 2): aqk(1, 2),
                (0, 3): av(4, 5), (0, 4): av(6, 7), (0, 5): aqk(2, 2),
                (0, 6): av(8, 9), (0, 7): av(10, 11), (0, 8): aqk(3, 2),
                (0, 9): av(12, 13), (0, 10): av(14, 15), (0, 11): aqk(0, 1),
                (0, 12): aqk(0, 3), (0, 14): aqk(1, 3),
                (1, 2): aqk(2, 3), (1, 5): aqk(3, 3),
                (1, 8): aqk(1, 0, 0), (1, 9): aqk(1, 0, 1),
                (1, 10): aqk(1, 0, 2), (1, 11): aqk(1, 0, 3),
                (2, 2): oj(0, 0), (2, 3): oj(0, 1), (2, 4): oj(0, 2),
                (2, 5): oj(0, 3), (2, 6): oj(0, 4), (2, 7): oj(0, 5),
                (2, 8): oj(0, 6), (2, 9): oj(0, 7),
                (2, 10): aqk(1, 1, 0), (2, 11): aqk(1, 1, 1),
                (2, 12): aqk(1, 1, 2), (2, 13): aqk(1, 1, 3),
                (3, 8): aqk(2, 0, 0), (3, 9): aqk(2, 0, 1),
                (3, 10): aqk(2, 0, 2), (3, 11): aqk(2, 0, 3),
                (4, 2): oj(1, 0), (4, 3): oj(1, 1), (4, 4): oj(1, 2),
                (4, 5): oj(1, 3), (4, 6): oj(1, 4), (4, 7): oj(1, 5),
                (4, 8): oj(1, 6), (4, 9): oj(1, 7),
                (4, 10): aqk(2, 1, 0), (4, 11): aqk(2, 1, 1),
                (4, 12): aqk(2, 1, 2), (4, 13): aqk(2, 1, 3),
                (5, 8): aqk(3, 0, 0), (5, 9): aqk(3, 0, 1),
                (5, 10): aqk(3, 0, 2), (5, 11): aqk(3, 0, 3),
                (6, 2): oj(2, 0), (6, 3): oj(2, 1), (6, 4): oj(2, 2),
                (6, 5): oj(2, 3), (6, 6): oj(2, 4), (6, 7): oj(2, 5),
                (6, 8): oj(2, 6), (6, 9): oj(2, 7),
                (6, 10): aqk(3, 1, 0), (6, 11): aqk(3, 1, 1),
                (6, 12): aqk(3, 1, 2), (6, 13): aqk(3, 1, 3),
                (7, 9): ojp(3, 0), (7, 11): ojp(3, 1),
                (7, 13): ojp(3, 2), (7, 15): ojp(3, 3),
            }

            # ---- main schedule ----
            phases = [(ic, pair) for ic in range(4) for pair in range(2)]

            NWARM = 14
            for w in range(NWARM):
                wacc = ps.tile([P, 512], F32, tag=btag(), name=f"warm_{w}")
                nc.tensor.matmul(wacc[:], wsrc[:, 0:128], wsrc[:],
                                 start=True, stop=True)

            emit_aqk_halves(0, 0)
            emit_aqk_halves(0, 2)
            s_cur = emit_scores(0, 0, 0)
            for pi, (ic, pair) in enumerate(phases):
                sset = pi % 2
                st8["other"] = 1 - sset
                pvl = [ps.tile([P, 512], F32, tag=f"pvl{sset}{hh}",
                               name=f"pvl_{pi}_{hh}") for hh in range(2)]
                vs_cur = vse_cur = None
                for jt in range(16):
                    if jt == 15:
                        nxt = ((phases[pi + 1][0], phases[pi + 1][1], 0)
                               if pi < 7 else None)
                    else:
                        nxt = (ic, pair, jt + 1)
                    if jt == 15:
                        # rank-1 first so PV(15) carries the stop and
                        # normalization starts immediately
                        emit_void_pvl(pair, pvl, vse_cur, stop=False)
                    s_cur = emit_exp_pvl(ic, pair, jt, s_cur, pvl, nxt,
                                         hooks.get((pi, jt)),
                                         pvstop=True)
                    if jt == 1:
                        vs_cur = emit_void_scores(ic, pair)
                    elif jt == 3:
                        vse_cur = emit_void_exp(vs_cur, ic, pair)
                if pi == 7:
                    last_pvl = pvl
                else:
                    osbs[(ic, pair)] = emit_norm(ic, pair, pvl)
            # tail: interleave last-phase norm chunks with the outproj fins
            osb1 = wp.tile([P, 512], BF16, tag="osb1", bufs=2, name="osb_3_1")
            for it in range(4):
                emit_norm_chunk(3, 1, last_pvl, osb1, it)
                emit_outproj_fin(3, it, osb1, yps_pre[it])

    nc.compile()
    return nc


def _voidvo(vv4):
    """[v_h | ones] rank-1 lhsT rows for the void value: [hh, pair, 128]."""
    out = np.ones((2, 2, P), np.float32)
    for pair in range(2):
        for hh in range(2):
            out[hh, pair, 0:D] = vv4[2 * pair + hh]
    return out


def _prep_inputs(x, w_qkv, w_out, b_out, void_q, void_k, void_v,
                 attention_trace, temperature_factor):
    """Host-side sharding / layout prep. Returns in_maps for 8 cores."""
    import ml_dtypes
    BF = ml_dtypes.bfloat16

    temp = np.maximum(1.0 + np.abs(attention_trace) * temperature_factor,
                      1.0).reshape(HEADS).astype(np.float32)
    scale = (DIM ** -0.5) / temp                       # [16] per head
    qcol_scale = np.repeat(scale, D)                   # [1024]
    wq_scaled = (w_qkv[:, 0:DIM] * qcol_scale[None, :]).astype(np.float32)
    wk = w_qkv[:, DIM:2 * DIM]
    wv_full = w_qkv[:, 2 * DIM:3 * DIM]
    vk = void_k.reshape(HEADS, D)
    vv = void_v.reshape(HEADS, D)

    in_maps = []
    for core in range(8):
        b, hg = divmod(core, 4)
        h0 = hg * HPC
        cs = slice(h0 * D, (h0 + HPC) * D)             # 256 feature cols
        in_maps.append({
            "xT": np.ascontiguousarray(x[b].T).astype(BF),
            "wqkv": np.ascontiguousarray(
                np.concatenate([wq_scaled[:, cs], wk[:, cs],
                                wv_full[:, cs]], axis=1)).astype(BF),
            "wout": np.ascontiguousarray(w_out[cs, :]).astype(BF),
            "voidk": np.ascontiguousarray(vk[h0:h0 + HPC].reshape(2, P)),
            "voidvo": _voidvo(vv[h0:h0 + HPC]),
        })
    return in_maps


def _run(in_maps, trace=False):
    from concourse import bass_utils
    if "nc" not in _cache:
        _cache["nc"] = _build()
    return bass_utils.run_bass_kernel_spmd(
        _cache["nc"], in_maps, core_ids=list(range(8)), trace=trace)


def kernel(x, w_qkv, w_out, b_out, void_q, void_k, void_v,
           attention_trace, temperature_factor):
    args = [np.asarray(a, dtype=np.float32) for a in
            (x, w_qkv, w_out, b_out, void_q, void_k, void_v,
             attention_trace, temperature_factor)]
    in_maps = _prep_inputs(*args)
    res = _run(in_maps)
    out = np.zeros((B, N, DIM), np.float32)
    for core in range(8):
        b = core // 4
        out[b] += np.asarray(res.results[core]["y"], dtype=np.float32)
    out += args[3][None, None, :]                      # b_out
    return out


# revision 44
# speedup vs baseline: 1.3357x; 1.0273x over previous
"""BlanchotianAttention TRN2 kernel: 8 NeuronCores, data-parallel over batch (2)
x tensor-parallel over heads (4 heads/core).

Design (pair-phased schedule, cost-model driven):
  - Host ships xT/wqkv/wout as bf16; DMA lands directly in the matmul operand
    tiles (no fp32 staging or rounding copies). Whole-tensor DMAs via
    "(ko p) c -> p ko c" rearranges: one descriptor batch per issue.
  - A chain of zero-data warmup matmuls keeps the PE p-state ramping while
    the input DMA lands, so real matmuls run at full clock from the start.
  - Stage A (qkv projection) in bf16; outputs copied PSUM->SBUF as f32r:
    QT/KT in [d, seq] head-pair tiles, V_aug [seq, v|ones] per j-tile,
    q pre-scaled by dim^-0.5/temperature on host.
  - Main loop: 8 phases = (i-chunk 0..3) x (head pair 0..1), 16 j-tiles each.
    Per (phase, jt): 2 score matmuls -> one [128,1024] exp on ACT -> 2 PV
    matmuls accumulating [v|ones] into the phase's pvl bank-set. PSUM: 2
    alternating score tiles (2 banks each) + 2 pvl bank-sets (2 banks each)
    = 8 banks; the set idle in a phase is borrowed by stage-A accumulators,
    outproj tiles and the void pipeline. Stage A / outproj work is drip-fed
    through per-jt hooks sized to the ACT slack per iteration.
  - Void token (j-tile 17) is handled as a rank-1 update instead of a full
    tile: tiny [128,16] S-layout score matmuls (fp32r needs even N, so each
    score is computed twice), a [128,16] exp, PE transpose, and a DRAM-bounce
    repack to [1,512] rows on partitions 0/64 (the only legal matmul operand
    bases); [v_h|ones] x exp(s_void) then closes each pvl accumulation.
    Each phase's void pipeline runs inside the previous phase.
  - Normalize = reciprocal + multiply on DVE (single PSUM operand per
    instruction; GPSIMD cannot touch PSUM). The last phase's norm is
    column-chunked and interleaved with the tail out-projection.
  - Out projection in bf16 (osb bf16 x wout bf16): per-oc PSUM tiles on
    borrowed banks, DVE/ACT copies, half-width bf16 y DMAs. The last
    i-chunk's pair0 matmuls are pre-started inside the final phase.
  - y partials are bf16; host sums partials in fp32 and adds b_out.

Timeline-sim: 194.7us (baseline 260.1us); rel err vs reference ~4.2e-3.
"""
import sys

sys.path.insert(0, "/opt/trn_rl_repo")

import numpy as np

DIM, HEADS, B, N = 1024, 16, 2, 2048
D = DIM // HEADS          # 64
HPC = HEADS // 4          # heads per core = 4
P = 128
KO = DIM // P             # 8 k-tiles

_cache = {}


def _build():
    import concourse.mybir as mybir
    import concourse.tile as tile
    from concourse import bacc

    F32 = mybir.dt.float32
    F32R = mybir.dt.float32r
    BF16 = mybir.dt.bfloat16
    Exp = mybir.ActivationFunctionType.Exp

    nc = bacc.Bacc("TRN2", target_bir_lowering=False, debug=False)
    xT = nc.dram_tensor("xT", [DIM, N], BF16, kind="ExternalInput").ap()
    wqkv = nc.dram_tensor("wqkv", [DIM, 768], BF16, kind="ExternalInput").ap()
    wout = nc.dram_tensor("wout", [256, DIM], BF16, kind="ExternalInput").ap()
    voidk = nc.dram_tensor("voidk", [2, P], F32, kind="ExternalInput").ap()
    voidvo = nc.dram_tensor("voidvo", [2, 2, P], BF16,
                            kind="ExternalInput").ap()
    ident_in = nc.dram_tensor("ident_in", [P, P], F32R,
                              kind="ExternalInput").ap()
    y = nc.dram_tensor("y", [N, DIM], BF16, kind="ExternalOutput").ap()
    vscr = nc.dram_tensor("vscr", [8, 16, P], BF16, kind="Internal").ap()

    xT_r = xT.rearrange("(ko p) s -> p ko s", p=P)
    wqkv_r = wqkv.rearrange("(ko p) c -> p ko c", p=P)
    wout_r = wout.rearrange("(k p) c -> p k c", p=P)

    with tile.TileContext(nc) as tc:
        with tc.tile_pool(name="persist", bufs=1) as pp, \
             tc.tile_pool(name="work", bufs=1) as wp, \
             tc.tile_pool(name="psum", bufs=1, space="PSUM") as ps:

            # ---- persistent SBUF ----
            xT_bf = pp.tile([P, KO, N], BF16)
            wqkv_bf = pp.tile([P, KO, 768], BF16)
            wout_bf = pp.tile([P, 2, DIM], BF16)
            qt = pp.tile([P, 2, N], F32R)
            kt = pp.tile([P, 2, 2048], F32R)
            va = pp.tile([P, 16, 512], F32R)
            ones = pp.tile([P, D], F32)
            vkt = pp.tile([P, 2], F32)
            vktr = pp.tile([P, 2, 2], F32R)   # void key, column doubled
            ident = pp.tile([P, P], F32R)
            vones = pp.tile([P, 2, P], BF16)   # [v_h | ones] rank-1 lhsT

            # ---- DMA issues (all SP queue; priority order) ----
            nc.sync.dma_start(wqkv_bf[:, :, 0:256], wqkv_r[:, :, 0:256])
            nc.sync.dma_start(xT_bf[:, :, 0:256], xT_r[:, :, 0:256])
            nc.sync.dma_start(wqkv_bf[:, :, 256:512], wqkv_r[:, :, 256:512])
            nc.sync.dma_start(xT_bf[:, :, 256:512], xT_r[:, :, 256:512])
            nc.sync.dma_start(wqkv_bf[:, :, 512:768], wqkv_r[:, :, 512:768])
            nc.sync.dma_start(xT_bf[:, :, 512:1024], xT_r[:, :, 512:1024])
            nc.sync.dma_start(xT_bf[:, :, 1024:1536], xT_r[:, :, 1024:1536])
            nc.sync.dma_start(xT_bf[:, :, 1536:2048], xT_r[:, :, 1536:2048])
            nc.sync.dma_start(wout_bf[:], wout_r)
            nc.sync.dma_start(vkt[:], voidk.rearrange("a p -> p a"))
            nc.sync.dma_start(vones[0:1, :, :], voidvo[0:1, :, :])
            nc.sync.dma_start(vones[64:65, :, :], voidvo[1:2, :, :])
            nc.sync.dma_start(ident[:], ident_in)

            # ---- setup on Pool (keeps DVE free for stage-A copies) ----
            nc.gpsimd.memset(ones[:], 1.0)
            nc.gpsimd.tensor_copy(
                vktr[:], vkt[:, :, None].to_broadcast([P, 2, 2]))
            for jt in range(16):
                nc.gpsimd.tensor_copy(
                    va[:, jt, :].rearrange("p (h c) -> p h c", c=P)[:, :, D:P],
                    ones[:, None, :].to_broadcast([P, 4, D]))

            # ---- PE pipeline warmup: zero-data matmuls keep the tensor
            # engine busy (and its p-state ramping) while input DMA lands ----
            wsrc = pp.tile([P, 512], BF16)
            nc.vector.memset(wsrc[:].bitcast(mybir.dt.uint16), 0)

            # ---- stage A (borrows the idle pvl bank-set) ----
            st8 = {"other": 1, "slot": 0}

            def btag():
                t = f"pvl{st8['other']}{st8['slot']}"
                st8["slot"] ^= 1
                return t

            aqk_accs = {}

            def emit_aqk_part(sc, ft, part, nparts=4):
                kpp = KO // nparts
                if part == 0:
                    aqk_accs[(sc, ft)] = ps.tile([P, 512], F32, tag=btag(),
                                                 name=f"aqk_{sc}_{ft}")
                acc = aqk_accs[(sc, ft)]
                for ko in range(part * kpp, (part + 1) * kpp):
                    nc.tensor.matmul(
                        acc[:],
                        wqkv_bf[:, ko, ft * P:(ft + 1) * P],
                        xT_bf[:, ko, sc * 512:(sc + 1) * 512],
                        start=(ko == 0), stop=(ko == KO - 1),
                    )
                if part == nparts - 1:
                    if ft < 2:
                        nc.vector.tensor_copy(
                            qt[:, ft, sc * 512:(sc + 1) * 512], acc[:])
                    else:
                        nc.vector.tensor_copy(
                            kt[:, ft - 2, sc * 512:(sc + 1) * 512], acc[:])

            def emit_aqk(sc, ft):
                for part in range(4):
                    emit_aqk_part(sc, ft, part)

            def emit_aqk_halves(sc, ft):
                # startup variant: accumulate each 256-col half separately so
                # compute starts as soon as the first half-chunk of xT lands
                acc = ps.tile([P, 512], F32, tag=btag(), name=f"aqkh_{sc}_{ft}")
                for half in range(2):
                    cs = slice(sc * 512 + half * 256, sc * 512 + half * 256 + 256)
                    for ko in range(KO):
                        nc.tensor.matmul(
                            acc[:, half * 256:(half + 1) * 256],
                            wqkv_bf[:, ko, ft * P:(ft + 1) * P],
                            xT_bf[:, ko, cs],
                            start=(ko == 0), stop=(ko == KO - 1),
                        )
                if ft < 2:
                    nc.vector.tensor_copy(qt[:, ft, sc * 512:(sc + 1) * 512],
                                          acc[:])
                else:
                    nc.vector.tensor_copy(kt[:, ft - 2, sc * 512:(sc + 1) * 512],
                                          acc[:])

            def emit_av(st):
                acc = ps.tile([P, 512], F32, tag=btag(), name=f"av_{st}")
                for ko in range(KO):
                    nc.tensor.matmul(
                        acc[:, 0:256],
                        xT_bf[:, ko, st * P:(st + 1) * P],
                        wqkv_bf[:, ko, 512:768],
                        start=(ko == 0), stop=(ko == KO - 1),
                    )
                nc.vector.tensor_copy(
                    va[:, st, :].rearrange("p (h c) -> p h c", c=P)[:, :, 0:D],
                    acc[:, 0:256].rearrange("p (h c) -> p h c", c=D))

            # ---- main loop pieces ----
            sidx = {"i": 0}

            def emit_scores(ic, pair, jt):
                isl = slice(ic * 512, (ic + 1) * 512)
                jsl = slice(jt * P, (jt + 1) * P)
                i = sidx["i"]
                sidx["i"] += 1
                s = ps.tile([P, 1024], F32, tag=f"s{i % 2}",
                            name=f"s_{ic}_{pair}_{jt}")
                for hh in range(2):
                    nc.tensor.matmul(
                        s[:, hh * 512:(hh + 1) * 512],
                        kt[hh * D:(hh + 1) * D, pair, jsl],
                        qt[hh * D:(hh + 1) * D, pair, isl],
                        start=True, stop=True)
                return s

            def emit_exp_pvl(ic, pair, jt, s_cur, pvl, nxt, hook,
                             pvstop=False):
                p = wp.tile([P, 1024], F32R, tag="pexp", bufs=6,
                            name=f"p_{ic}_{pair}_{jt}")
                nc.scalar.activation(p[:], s_cur[:], Exp)
                s_nxt = emit_scores(*nxt) if nxt is not None else None
                if hook is not None:
                    hook()
                for hh in range(2):
                    h = 2 * pair + hh
                    nc.tensor.matmul(
                        pvl[hh][:],
                        va[:, jt, h * P:(h + 1) * P],
                        p[:, hh * 512:(hh + 1) * 512],
                        start=(jt == 0), stop=(jt == 15 and pvstop),
                    )
                return s_nxt

            # void key: S-layout scores [128 i-rows, 8 = (head, i-subtile)],
            # a tiny [128,8] exp, PE transpose to [8,128], then rank-1
            # [v|ones] x exp(s_void) closes the pvl accumulation
            def emit_void_scores(ic, pair):
                # fp32r matmuls need an even moving dim: compute each void
                # score twice (doubled key column), use the even columns
                vs = ps.tile([P, 512], F32, tag=btag(), name=f"vs_{ic}_{pair}")
                for hh in range(2):
                    for sub in range(4):
                        c = 2 * (hh * 4 + sub)
                        nc.tensor.matmul(
                            vs[:, c:c + 2],
                            qt[hh * D:(hh + 1) * D, pair,
                               ic * 512 + sub * P: ic * 512 + (sub + 1) * P],
                            vktr[hh * D:(hh + 1) * D, pair, :],
                            start=True, stop=True)
                return vs

            def emit_void_exp(vs, ic, pair):
                vse8 = wp.tile([P, 16], F32R, tag="vse8", bufs=2,
                               name=f"vse8_{ic}_{pair}")
                nc.scalar.activation(vse8[:], vs[:, 0:16], Exp)
                vst = ps.tile([P, 512], F32R, tag=btag(),
                              name=f"vst_{ic}_{pair}")
                nc.tensor.transpose(vst[0:16, 0:P], vse8[:], ident[:])
                vstb = wp.tile([16, P], BF16, tag="vstb", bufs=2,
                               name=f"vstb_{ic}_{pair}")
                nc.vector.tensor_copy(vstb[:], vst[0:16, 0:P])
                # repack the 8 rows into [1,512] rows on partitions 0 / 64
                # (the only legal matmul operand bases), bouncing through a
                # DRAM scratch slot (per phase, so no cross-phase hazard)
                pi = 2 * ic + pair
                nc.sync.dma_start(vscr[pi], vstb[:])
                vsty = wp.tile([P, 512], BF16, tag="vsty", bufs=2,
                               name=f"vsty_{ic}_{pair}")
                vscr_r = vscr[pi].rearrange(
                    "(x s t) c -> x t s c", x=2, t=2)[:, 0, :, :]
                for hh in range(2):
                    nc.sync.dma_start(
                        vsty[hh * D:hh * D + 1, :].rearrange(
                            "p (s c) -> p s c", c=P),
                        vscr_r[hh:hh + 1, :, :])
                return vsty

            def emit_void_pvl(pair, pvl, vsty, stop):
                for hh in range(2):
                    nc.tensor.matmul(
                        pvl[hh][:],
                        vones[hh * D:hh * D + 1, pair, :],
                        vsty[hh * D:hh * D + 1, :],
                        start=False, stop=stop)

            def emit_norm_chunk(ic, pair, pvl, osb, chunk):
                cs = slice(chunk * P, (chunk + 1) * P)
                for hh in range(2):
                    r_sb = wp.tile([D, P], F32, tag=f"rsbs{hh}", bufs=2,
                                   name=f"rsbs_{ic}_{pair}_{hh}_{chunk}")
                    nc.vector.reciprocal(r_sb[:], pvl[hh][D:P, cs])
                    nc.vector.tensor_tensor(
                        osb[hh * D:(hh + 1) * D, cs],
                        pvl[hh][0:D, cs], r_sb[:],
                        mybir.AluOpType.mult)

            def emit_norm(ic, pair, pvl):
                # BIR allows only one PSUM operand per instruction: move one
                # side to SBUF first. head0 via DVE recip+mult, head1 via
                # Pool copy+divide.
                osb = wp.tile([P, 512], BF16, tag=f"osb{pair}", bufs=2,
                              name=f"osb_{ic}_{pair}")
                for hh in range(2):
                    r_sb = wp.tile([D, 512], F32, tag=f"rsb{hh}", bufs=2,
                                   name=f"rsb_{ic}_{pair}_{hh}")
                    nc.vector.reciprocal(r_sb[:], pvl[hh][D:P, :])
                    nc.vector.tensor_tensor(osb[hh * D:(hh + 1) * D, :],
                                            pvl[hh][0:D, :], r_sb[:],
                                            mybir.AluOpType.mult)
                return osb

            def emit_outproj_oc(ic, it, oc, osbs):
                yp = ps.tile([P, 512], F32, tag=btag(),
                             name=f"y_{ic}_{it}_{oc}")
                for pair in range(2):
                    nc.tensor.matmul(
                        yp[:],
                        osbs[pair][:, it * P:(it + 1) * P],
                        wout_bf[:, pair, oc * 512:(oc + 1) * 512],
                        start=(pair == 0), stop=(pair == 1),
                    )
                ysbh = wp.tile([P, 512], BF16, tag="ysbh", bufs=4,
                               name=f"ysbh_{ic}_{it}_{oc}")
                nc.vector.tensor_copy(ysbh[:], yp[:])
                nc.sync.dma_start(
                    y[ic * 512 + it * P: ic * 512 + (it + 1) * P,
                      oc * 512:(oc + 1) * 512], ysbh[:])

            def emit_outproj_pre(ic, it, osb0):
                yps = []
                for oc in range(2):
                    yp = ps.tile([P, 512], F32, tag=btag(),
                                 name=f"y_{ic}_{it}_{oc}")
                    nc.tensor.matmul(
                        yp[:],
                        osb0[:, it * P:(it + 1) * P],
                        wout_bf[:, 0, oc * 512:(oc + 1) * 512],
                        start=True, stop=False,
                    )
                    yps.append(yp)
                return yps

            def emit_outproj_fin(ic, it, osb1, yps):
                for oc in range(2):
                    nc.tensor.matmul(
                        yps[oc][:],
                        osb1[:, it * P:(it + 1) * P],
                        wout_bf[:, 1, oc * 512:(oc + 1) * 512],
                        start=False, stop=True,
                    )
                    ysbh = wp.tile([P, 512], BF16, tag="ysbh", bufs=4,
                                   name=f"ysbh_{ic}_{it}_{oc}")
                    if oc == 0:
                        nc.vector.tensor_copy(ysbh[:], yps[oc][:])
                    else:
                        nc.scalar.copy(ysbh[:], yps[oc][:])
                    nc.sync.dma_start(
                        y[ic * 512 + it * P: ic * 512 + (it + 1) * P,
                          oc * 512:(oc + 1) * 512], ysbh[:])

            # ---- hook schedule ----
            osbs = {}
            yps_pre = {}

            def oj(ic, k):
                def f():
                    emit_outproj_oc(ic, k // 2, k % 2,
                                    [osbs[(ic, 0)], osbs[(ic, 1)]])
                return f

            def ojp(ic, it):
                def f():
                    yps_pre[it] = emit_outproj_pre(ic, it, osbs[(ic, 0)])
                return f

            def aqk(sc, ft, part=None):
                if part is None:
                    return lambda: emit_aqk(sc, ft)
                return lambda: emit_aqk_part(sc, ft, part)

            def av(*sts):
                return lambda: [emit_av(st) for st in sts]

            hooks = {
                (0, 0): av(0, 1), (0, 1): av(2, 3), (0, 2): aqk(1, 2),
                (0, 3): av(4, 5), (0, 4): av(6, 7), (0, 5): aqk(2, 2),
                (0, 6): av(8, 9), (0, 7): av(10, 11), (0, 8): aqk(3, 2),
                (0, 9): av(12, 13), (0, 10): av(14, 15), (0, 11): aqk(0, 1),
                (0, 12): aqk(0, 3), (0, 14): aqk(1, 3),
                (1, 2): aqk(2, 3, 0), (1, 3): aqk(2, 3, 1),
                (1, 4): aqk(2, 3, 2), (1, 5): aqk(2, 3, 3),
                (1, 6): aqk(3, 3, 0), (1, 7): aqk(3, 3, 1),
                (1, 8): aqk(3, 3, 2), (1, 9): aqk(3, 3, 3),
                (1, 10): aqk(1, 0, 0), (1, 11): aqk(1, 0, 1),
                (1, 12): aqk(1, 0, 2), (1, 13): aqk(1, 0, 3),
                (2, 2): oj(0, 0), (2, 3): oj(0, 1), (2, 4): oj(0, 2),
                (2, 5): oj(0, 3), (2, 6): oj(0, 4), (2, 7): oj(0, 5),
                (3, 2): oj(0, 6), (3, 3): oj(0, 7),
                (2, 10): aqk(1, 1, 0), (2, 11): aqk(1, 1, 1),
                (2, 12): aqk(1, 1, 2), (2, 13): aqk(1, 1, 3),
                (3, 8): aqk(2, 0, 0), (3, 9): aqk(2, 0, 1),
                (3, 10): aqk(2, 0, 2), (3, 11): aqk(2, 0, 3),
                (4, 2): oj(1, 0), (4, 3): oj(1, 1), (4, 4): oj(1, 2),
                (4, 5): oj(1, 3), (4, 6): oj(1, 4), (4, 7): oj(1, 5),
                (5, 2): oj(1, 6), (5, 3): oj(1, 7),
                (4, 10): aqk(2, 1, 0), (4, 11): aqk(2, 1, 1),
                (4, 12): aqk(2, 1, 2), (4, 13): aqk(2, 1, 3),
                (5, 8): aqk(3, 0, 0), (5, 9): aqk(3, 0, 1),
                (5, 10): aqk(3, 0, 2), (5, 11): aqk(3, 0, 3),
                (6, 2): oj(2, 0), (6, 3): oj(2, 1), (6, 4): oj(2, 2),
                (6, 5): oj(2, 3), (6, 6): oj(2, 4), (6, 7): oj(2, 5),
                (7, 2): oj(2, 6), (7, 3): oj(2, 7),
                (6, 10): aqk(3, 1, 0), (6, 11): aqk(3, 1, 1),
                (6, 12): aqk(3, 1, 2), (6, 13): aqk(3, 1, 3),
                (7, 9): ojp(3, 0), (7, 11): ojp(3, 1),
                (7, 13): ojp(3, 2), (7, 15): ojp(3, 3),
            }

            # ---- main schedule ----
            phases = [(ic, pair) for ic in range(4) for pair in range(2)]

            NWARM = 14
            for w in range(NWARM):
                wacc = ps.tile([P, 512], F32, tag=btag(), name=f"warm_{w}")
                nc.tensor.matmul(wacc[:], wsrc[:, 0:128], wsrc[:],
                                 start=True, stop=True)

            emit_aqk_halves(0, 0)
            emit_aqk_halves(0, 2)
            s_cur = emit_scores(0, 0, 0)
            vse_cur = vse_nxt = None
            for pi, (ic, pair) in enumerate(phases):
                sset = pi % 2
                st8["other"] = 1 - sset
                pvl = [ps.tile([P, 512], F32, tag=f"pvl{sset}{hh}",
                               name=f"pvl_{pi}_{hh}") for hh in range(2)]
                vs_cur = None
                for jt in range(16):
                    if jt == 15:
                        nxt = ((phases[pi + 1][0], phases[pi + 1][1], 0)
                               if pi < 7 else None)
                    else:
                        nxt = (ic, pair, jt + 1)
                    if jt == 15:
                        # rank-1 first so PV(15) carries the stop and
                        # normalization starts immediately
                        emit_void_pvl(pair, pvl, vse_cur, stop=False)
                    s_cur = emit_exp_pvl(ic, pair, jt, s_cur, pvl, nxt,
                                         hooks.get((pi, jt)),
                                         pvstop=True)
                    if pi == 0:
                        if jt == 1:
                            vs_cur = emit_void_scores(ic, pair)
                        elif jt == 3:
                            vse_cur = emit_void_exp(vs_cur, ic, pair)
                    if jt == 0 and pi in (2, 4, 6):
                        # PE-bound phase: void exp after jt0 so PV(0) isn't
                        # delayed behind it at the boundary
                        vse_cur = emit_void_exp(vs_nxt_pend, ic, pair)
                    if jt == 13 and pi < 7:
                        vs_nxt = emit_void_scores(*phases[pi + 1])
                vs_nxt_pend = None
                if pi < 7:
                    if pi + 1 in (2, 4, 6):
                        vs_nxt_pend = vs_nxt      # exp deferred into pi+1
                        vse_nxt = None
                    else:
                        # ACT-bound next phase: void exp fills the boundary
                        # ACT bubble
                        vse_nxt = emit_void_exp(vs_nxt, *phases[pi + 1])
                if pi == 7:
                    last_pvl = pvl
                else:
                    osbs[(ic, pair)] = emit_norm(ic, pair, pvl)
                vse_cur = vse_nxt
            # tail: interleave last-phase norm chunks with the outproj fins
            osb1 = wp.tile([P, 512], BF16, tag="osb1", bufs=2, name="osb_3_1")
            for it in range(4):
                emit_norm_chunk(3, 1, last_pvl, osb1, it)
                emit_outproj_fin(3, it, osb1, yps_pre[it])

    nc.compile()
    return nc


def _voidvo(vv4):
    """[v_h | ones] rank-1 lhsT rows for the void value: [hh, pair, 128]."""
    import ml_dtypes
    out = np.ones((2, 2, P), np.float32)
    for pair in range(2):
        for hh in range(2):
            out[hh, pair, 0:D] = vv4[2 * pair + hh]
    return out.astype(ml_dtypes.bfloat16)


def _prep_inputs(x, w_qkv, w_out, b_out, void_q, void_k, void_v,
                 attention_trace, temperature_factor):
    """Host-side sharding / layout prep. Returns in_maps for 8 cores."""
    import ml_dtypes
    BF = ml_dtypes.bfloat16

    temp = np.maximum(1.0 + np.abs(attention_trace) * temperature_factor,
                      1.0).reshape(HEADS).astype(np.float32)
    scale = (DIM ** -0.5) / temp                       # [16] per head
    qcol_scale = np.repeat(scale, D)                   # [1024]
    wq_scaled = (w_qkv[:, 0:DIM] * qcol_scale[None, :]).astype(np.float32)
    wk = w_qkv[:, DIM:2 * DIM]
    wv_full = w_qkv[:, 2 * DIM:3 * DIM]
    vk = void_k.reshape(HEADS, D)
    vv = void_v.reshape(HEADS, D)

    in_maps = []
    for core in range(8):
        b, hg = divmod(core, 4)
        h0 = hg * HPC
        cs = slice(h0 * D, (h0 + HPC) * D)             # 256 feature cols
        in_maps.append({
            "xT": np.ascontiguousarray(x[b].T).astype(BF),
            "wqkv": np.ascontiguousarray(
                np.concatenate([wq_scaled[:, cs], wk[:, cs],
                                wv_full[:, cs]], axis=1)).astype(BF),
            "wout": np.ascontiguousarray(w_out[cs, :]).astype(BF),
            "voidk": np.ascontiguousarray(vk[h0:h0 + HPC].reshape(2, P)),
            "voidvo": _voidvo(vv[h0:h0 + HPC]),
            "ident_in": np.eye(P, dtype=np.float32),
        })
    return in_maps


def _run(in_maps, trace=False):
    from concourse import bass_utils
    if "nc" not in _cache:
        _cache["nc"] = _build()
    return bass_utils.run_bass_kernel_spmd(
        _cache["nc"], in_maps, core_ids=list(range(8)), trace=trace)


def kernel(x, w_qkv, w_out, b_out, void_q, void_k, void_v,
           attention_trace, temperature_factor):
    args = [np.asarray(a, dtype=np.float32) for a in
            (x, w_qkv, w_out, b_out, void_q, void_k, void_v,
             attention_trace, temperature_factor)]
    in_maps = _prep_inputs(*args)
    res = _run(in_maps)
    out = np.zeros((B, N, DIM), np.float32)
    for core in range(8):
        b = core // 4
        out[b] += np.asarray(res.results[core]["y"], dtype=np.float32)
    out += args[3][None, None, :]                      # b_out
    return out


# revision 47
# speedup vs baseline: 1.3429x; 1.0054x over previous
"""BlanchotianAttention TRN2 kernel: 8 NeuronCores, data-parallel over batch (2)
x tensor-parallel over heads (4 heads/core).

Design (pair-phased schedule, cost-model driven):
  - Host ships xT/wqkv/wout as bf16; DMA lands directly in the matmul operand
    tiles (no fp32 staging or rounding copies). Whole-tensor DMAs via
    "(ko p) c -> p ko c" rearranges: one descriptor batch per issue.
  - A chain of zero-data warmup matmuls keeps the PE p-state ramping while
    the input DMA lands, so real matmuls run at full clock from the start.
  - Stage A (qkv projection) in bf16; outputs copied PSUM->SBUF as f32r:
    QT/KT in [d, seq] head-pair tiles, V_aug [seq, v|ones] per j-tile,
    q pre-scaled by dim^-0.5/temperature on host.
  - Main loop: 8 phases = (i-chunk 0..3) x (head pair 0..1), 16 j-tiles each.
    Per (phase, jt): 2 score matmuls -> one [128,1024] exp on ACT -> 2 PV
    matmuls accumulating [v|ones] into the phase's pvl bank-set. PSUM: 2
    alternating score tiles (2 banks each) + 2 pvl bank-sets (2 banks each)
    = 8 banks; the set idle in a phase is borrowed by stage-A accumulators,
    outproj tiles and the void pipeline. Stage A / outproj work is drip-fed
    through per-jt hooks sized to the ACT slack per iteration.
  - Void token (j-tile 17) is handled as a rank-1 update instead of a full
    tile: tiny [128,16] S-layout score matmuls (fp32r needs even N, so each
    score is computed twice), a [128,16] exp, PE transpose, and a DRAM-bounce
    repack to [1,512] rows on partitions 0/64 (the only legal matmul operand
    bases); [v_h|ones] x exp(s_void) then closes each pvl accumulation.
    Each phase's void pipeline runs inside the previous phase.
  - Normalize = reciprocal + multiply on DVE (single PSUM operand per
    instruction; GPSIMD cannot touch PSUM). The last phase's norm is
    column-chunked and interleaved with the tail out-projection.
  - Out projection in bf16 (osb bf16 x wout bf16): per-oc PSUM tiles on
    borrowed banks, DVE/ACT copies, half-width bf16 y DMAs. The last
    i-chunk's pair0 matmuls are pre-started inside the final phase.
  - y partials are bf16; host sums partials in fp32 and adds b_out.

Timeline-sim: 194.7us (baseline 260.1us); rel err vs reference ~4.2e-3.
"""
import sys

sys.path.insert(0, "/opt/trn_rl_repo")

import numpy as np

DIM, HEADS, B, N = 1024, 16, 2, 2048
D = DIM // HEADS          # 64
HPC = HEADS // 4          # heads per core = 4
P = 128
KO = DIM // P             # 8 k-tiles

_cache = {}


def _build():
    import concourse.mybir as mybir
    import concourse.tile as tile
    from concourse import bacc

    F32 = mybir.dt.float32
    F32R = mybir.dt.float32r
    BF16 = mybir.dt.bfloat16
    Exp = mybir.ActivationFunctionType.Exp

    nc = bacc.Bacc("TRN2", target_bir_lowering=False, debug=False)
    xT = nc.dram_tensor("xT", [DIM, N], BF16, kind="ExternalInput").ap()
    wqkv = nc.dram_tensor("wqkv", [DIM, 768], BF16, kind="ExternalInput").ap()
    wout = nc.dram_tensor("wout", [256, DIM], BF16, kind="ExternalInput").ap()
    voidk = nc.dram_tensor("voidk", [2, P], F32, kind="ExternalInput").ap()
    voidvo = nc.dram_tensor("voidvo", [2, 2, P], BF16,
                            kind="ExternalInput").ap()
    ident_in = nc.dram_tensor("ident_in", [P, P], F32R,
                              kind="ExternalInput").ap()
    y = nc.dram_tensor("y", [N, DIM], BF16, kind="ExternalOutput").ap()
    vscr = nc.dram_tensor("vscr", [8, 16, P], BF16, kind="Internal").ap()

    xT_r = xT.rearrange("(ko p) s -> p ko s", p=P)
    wqkv_r = wqkv.rearrange("(ko p) c -> p ko c", p=P)
    wout_r = wout.rearrange("(k p) c -> p k c", p=P)

    with tile.TileContext(nc) as tc:
        with tc.tile_pool(name="persist", bufs=1) as pp, \
             tc.tile_pool(name="work", bufs=1) as wp, \
             tc.tile_pool(name="psum", bufs=1, space="PSUM") as ps:

            # ---- persistent SBUF ----
            xT_bf = pp.tile([P, KO, N], BF16)
            wqkv_bf = pp.tile([P, KO, 768], BF16)
            wout_bf = pp.tile([P, 2, DIM], BF16)
            qt = pp.tile([P, 2, N], F32R)
            kt = pp.tile([P, 2, 2048], F32R)
            va = pp.tile([P, 16, 512], F32R)
            ones = pp.tile([P, D], F32)
            vkt = pp.tile([P, 2], F32)
            vktr = pp.tile([P, 2, 2], F32R)   # void key, column doubled
            ident = pp.tile([P, P], F32R)
            vones = pp.tile([P, 2, P], BF16)   # [v_h | ones] rank-1 lhsT

            # ---- DMA issues (all SP queue; priority order) ----
            nc.sync.dma_start(wqkv_bf[:, :, 0:256], wqkv_r[:, :, 0:256])
            nc.sync.dma_start(xT_bf[:, :, 0:256], xT_r[:, :, 0:256])
            nc.sync.dma_start(wqkv_bf[:, :, 256:512], wqkv_r[:, :, 256:512])
            nc.sync.dma_start(xT_bf[:, :, 256:512], xT_r[:, :, 256:512])
            nc.sync.dma_start(wqkv_bf[:, :, 512:768], wqkv_r[:, :, 512:768])
            nc.sync.dma_start(xT_bf[:, :, 512:1024], xT_r[:, :, 512:1024])
            nc.sync.dma_start(xT_bf[:, :, 1024:1536], xT_r[:, :, 1024:1536])
            nc.sync.dma_start(xT_bf[:, :, 1536:2048], xT_r[:, :, 1536:2048])
            nc.sync.dma_start(wout_bf[:], wout_r)
            nc.sync.dma_start(vkt[:], voidk.rearrange("a p -> p a"))
            nc.sync.dma_start(vones[0:1, :, :], voidvo[0:1, :, :])
            nc.sync.dma_start(vones[64:65, :, :], voidvo[1:2, :, :])
            nc.sync.dma_start(ident[:], ident_in)

            # ---- setup on Pool (keeps DVE free for stage-A copies) ----
            nc.gpsimd.memset(ones[:], 1.0)
            nc.gpsimd.tensor_copy(
                vktr[:], vkt[:, :, None].to_broadcast([P, 2, 2]))
            for jt in range(16):
                nc.gpsimd.tensor_copy(
                    va[:, jt, :].rearrange("p (h c) -> p h c", c=P)[:, :, D:P],
                    ones[:, None, :].to_broadcast([P, 4, D]))

            # ---- PE pipeline warmup: zero-data matmuls keep the tensor
            # engine busy (and its p-state ramping) while input DMA lands ----
            wsrc = pp.tile([P, 512], BF16)
            nc.vector.memset(wsrc[:].bitcast(mybir.dt.uint16), 0)

            # ---- stage A (borrows the idle pvl bank-set) ----
            st8 = {"other": 1, "slot": 0}

            def btag():
                t = f"pvl{st8['other']}{st8['slot']}"
                st8["slot"] ^= 1
                return t

            aqk_accs = {}

            def emit_aqk_part(sc, ft, part, nparts=4):
                kpp = KO // nparts
                if part == 0:
                    aqk_accs[(sc, ft)] = ps.tile([P, 512], F32, tag=btag(),
                                                 name=f"aqk_{sc}_{ft}")
                acc = aqk_accs[(sc, ft)]
                for ko in range(part * kpp, (part + 1) * kpp):
                    nc.tensor.matmul(
                        acc[:],
                        wqkv_bf[:, ko, ft * P:(ft + 1) * P],
                        xT_bf[:, ko, sc * 512:(sc + 1) * 512],
                        start=(ko == 0), stop=(ko == KO - 1),
                    )
                if part == nparts - 1:
                    if ft < 2:
                        nc.vector.tensor_copy(
                            qt[:, ft, sc * 512:(sc + 1) * 512], acc[:])
                    else:
                        nc.vector.tensor_copy(
                            kt[:, ft - 2, sc * 512:(sc + 1) * 512], acc[:])

            def emit_aqk(sc, ft):
                for part in range(4):
                    emit_aqk_part(sc, ft, part)

            def emit_aqk_halves(sc, ft):
                # startup variant: accumulate each 256-col half separately so
                # compute starts as soon as the first half-chunk of xT lands
                acc = ps.tile([P, 512], F32, tag=btag(), name=f"aqkh_{sc}_{ft}")
                for half in range(2):
                    cs = slice(sc * 512 + half * 256, sc * 512 + half * 256 + 256)
                    for ko in range(KO):
                        nc.tensor.matmul(
                            acc[:, half * 256:(half + 1) * 256],
                            wqkv_bf[:, ko, ft * P:(ft + 1) * P],
                            xT_bf[:, ko, cs],
                            start=(ko == 0), stop=(ko == KO - 1),
                        )
                if ft < 2:
                    nc.vector.tensor_copy(qt[:, ft, sc * 512:(sc + 1) * 512],
                                          acc[:])
                else:
                    nc.vector.tensor_copy(kt[:, ft - 2, sc * 512:(sc + 1) * 512],
                                          acc[:])

            def emit_av(st):
                acc = ps.tile([P, 512], F32, tag=btag(), name=f"av_{st}")
                for ko in range(KO):
                    nc.tensor.matmul(
                        acc[:, 0:256],
                        xT_bf[:, ko, st * P:(st + 1) * P],
                        wqkv_bf[:, ko, 512:768],
                        start=(ko == 0), stop=(ko == KO - 1),
                    )
                nc.vector.tensor_copy(
                    va[:, st, :].rearrange("p (h c) -> p h c", c=P)[:, :, 0:D],
                    acc[:, 0:256].rearrange("p (h c) -> p h c", c=D))

            # ---- main loop pieces ----
            sidx = {"i": 0}

            def emit_scores(ic, pair, jt):
                isl = slice(ic * 512, (ic + 1) * 512)
                jsl = slice(jt * P, (jt + 1) * P)
                i = sidx["i"]
                sidx["i"] += 1
                s = ps.tile([P, 1024], F32, tag=f"s{i % 2}",
                            name=f"s_{ic}_{pair}_{jt}")
                for hh in range(2):
                    nc.tensor.matmul(
                        s[:, hh * 512:(hh + 1) * 512],
                        kt[hh * D:(hh + 1) * D, pair, jsl],
                        qt[hh * D:(hh + 1) * D, pair, isl],
                        start=True, stop=True)
                return s

            def emit_exp_pvl(ic, pair, jt, s_cur, pvl, nxt, hook,
                             pvstop=False):
                p = wp.tile([P, 1024], F32R, tag="pexp", bufs=6,
                            name=f"p_{ic}_{pair}_{jt}")
                nc.scalar.activation(p[:], s_cur[:], Exp)
                s_nxt = emit_scores(*nxt) if nxt is not None else None
                if hook is not None:
                    hook()
                for hh in range(2):
                    h = 2 * pair + hh
                    nc.tensor.matmul(
                        pvl[hh][:],
                        va[:, jt, h * P:(h + 1) * P],
                        p[:, hh * 512:(hh + 1) * 512],
                        start=(jt == 0), stop=(jt == 15 and pvstop),
                    )
                return s_nxt

            # void key: S-layout scores [128 i-rows, 8 = (head, i-subtile)],
            # a tiny [128,8] exp, PE transpose to [8,128], then rank-1
            # [v|ones] x exp(s_void) closes the pvl accumulation
            def emit_void_scores(ic, pair):
                # fp32r matmuls need an even moving dim: compute each void
                # score twice (doubled key column), use the even columns
                vs = ps.tile([P, 512], F32, tag=btag(), name=f"vs_{ic}_{pair}")
                for hh in range(2):
                    for sub in range(4):
                        c = 2 * (hh * 4 + sub)
                        nc.tensor.matmul(
                            vs[:, c:c + 2],
                            qt[hh * D:(hh + 1) * D, pair,
                               ic * 512 + sub * P: ic * 512 + (sub + 1) * P],
                            vktr[hh * D:(hh + 1) * D, pair, :],
                            start=True, stop=True)
                return vs

            def emit_void_exp(vs, ic, pair):
                vse8 = wp.tile([P, 16], F32R, tag="vse8", bufs=2,
                               name=f"vse8_{ic}_{pair}")
                nc.scalar.activation(vse8[:], vs[:, 0:16], Exp)
                vst = ps.tile([P, 512], F32R, tag=btag(),
                              name=f"vst_{ic}_{pair}")
                nc.tensor.transpose(vst[0:16, 0:P], vse8[:], ident[:])
                vstb = wp.tile([16, P], BF16, tag="vstb", bufs=2,
                               name=f"vstb_{ic}_{pair}")
                nc.vector.tensor_copy(vstb[:], vst[0:16, 0:P])
                # repack the 8 rows into [1,512] rows on partitions 0 / 64
                # (the only legal matmul operand bases), bouncing through a
                # DRAM scratch slot (per phase, so no cross-phase hazard)
                pi = 2 * ic + pair
                nc.sync.dma_start(vscr[pi], vstb[:])
                vsty = wp.tile([P, 512], BF16, tag="vsty", bufs=2,
                               name=f"vsty_{ic}_{pair}")
                vscr_r = vscr[pi].rearrange(
                    "(x s t) c -> x t s c", x=2, t=2)[:, 0, :, :]
                for hh in range(2):
                    nc.sync.dma_start(
                        vsty[hh * D:hh * D + 1, :].rearrange(
                            "p (s c) -> p s c", c=P),
                        vscr_r[hh:hh + 1, :, :])
                return vsty

            def emit_void_pvl(pair, pvl, vsty, stop):
                for hh in range(2):
                    nc.tensor.matmul(
                        pvl[hh][:],
                        vones[hh * D:hh * D + 1, pair, :],
                        vsty[hh * D:hh * D + 1, :],
                        start=False, stop=stop)

            def emit_norm_chunk(ic, pair, pvl, osb, chunk):
                cs = slice(chunk * P, (chunk + 1) * P)
                for hh in range(2):
                    r_sb = wp.tile([D, P], F32, tag=f"rsbs{hh}", bufs=2,
                                   name=f"rsbs_{ic}_{pair}_{hh}_{chunk}")
                    nc.vector.reciprocal(r_sb[:], pvl[hh][D:P, cs])
                    nc.vector.tensor_tensor(
                        osb[hh * D:(hh + 1) * D, cs],
                        pvl[hh][0:D, cs], r_sb[:],
                        mybir.AluOpType.mult)

            def emit_norm(ic, pair, pvl):
                # BIR allows only one PSUM operand per instruction: move one
                # side to SBUF first. head0 via DVE recip+mult, head1 via
                # Pool copy+divide.
                osb = wp.tile([P, 512], BF16, tag=f"osb{pair}", bufs=2,
                              name=f"osb_{ic}_{pair}")
                for hh in range(2):
                    r_sb = wp.tile([D, 512], F32, tag=f"rsb{hh}", bufs=2,
                                   name=f"rsb_{ic}_{pair}_{hh}")
                    nc.vector.reciprocal(r_sb[:], pvl[hh][D:P, :])
                    nc.vector.tensor_tensor(osb[hh * D:(hh + 1) * D, :],
                                            pvl[hh][0:D, :], r_sb[:],
                                            mybir.AluOpType.mult)
                return osb

            def emit_outproj_oc(ic, it, oc, osbs):
                yp = ps.tile([P, 512], F32, tag=btag(),
                             name=f"y_{ic}_{it}_{oc}")
                for pair in range(2):
                    nc.tensor.matmul(
                        yp[:],
                        osbs[pair][:, it * P:(it + 1) * P],
                        wout_bf[:, pair, oc * 512:(oc + 1) * 512],
                        start=(pair == 0), stop=(pair == 1),
                    )
                ysbh = wp.tile([P, 512], BF16, tag="ysbh", bufs=4,
                               name=f"ysbh_{ic}_{it}_{oc}")
                nc.vector.tensor_copy(ysbh[:], yp[:])
                nc.sync.dma_start(
                    y[ic * 512 + it * P: ic * 512 + (it + 1) * P,
                      oc * 512:(oc + 1) * 512], ysbh[:])

            def emit_outproj_pre(ic, it, osb0):
                yps = []
                for oc in range(2):
                    yp = ps.tile([P, 512], F32, tag=btag(),
                                 name=f"y_{ic}_{it}_{oc}")
                    nc.tensor.matmul(
                        yp[:],
                        osb0[:, it * P:(it + 1) * P],
                        wout_bf[:, 0, oc * 512:(oc + 1) * 512],
                        start=True, stop=False,
                    )
                    yps.append(yp)
                return yps

            def emit_outproj_fin(ic, it, osb1, yps):
                for oc in range(2):
                    nc.tensor.matmul(
                        yps[oc][:],
                        osb1[:, it * P:(it + 1) * P],
                        wout_bf[:, 1, oc * 512:(oc + 1) * 512],
                        start=False, stop=True,
                    )
                    ysbh = wp.tile([P, 512], BF16, tag="ysbh", bufs=4,
                                   name=f"ysbh_{ic}_{it}_{oc}")
                    if oc == 0:
                        nc.vector.tensor_copy(ysbh[:], yps[oc][:])
                    else:
                        nc.scalar.copy(ysbh[:], yps[oc][:])
                    nc.sync.dma_start(
                        y[ic * 512 + it * P: ic * 512 + (it + 1) * P,
                          oc * 512:(oc + 1) * 512], ysbh[:])

            # ---- hook schedule ----
            osbs = {}
            yps_pre = {}

            yp_mid = {}

            def ojh(ic, k, pair):
                def f():
                    it, oc = k // 2, k % 2
                    osbs_ = [osbs[(ic, 0)], osbs[(ic, 1)]]
                    if pair == 0:
                        yp_mid[(ic, k)] = ps.tile(
                            [P, 512], F32, tag=btag(),
                            name=f"y_{ic}_{it}_{oc}")
                    yp = yp_mid[(ic, k)]
                    nc.tensor.matmul(
                        yp[:],
                        osbs_[pair][:, it * P:(it + 1) * P],
                        wout_bf[:, pair, oc * 512:(oc + 1) * 512],
                        start=(pair == 0), stop=(pair == 1),
                    )
                    if pair == 1:
                        ysbh = wp.tile([P, 512], BF16, tag="ysbh", bufs=4,
                                       name=f"ysbh_{ic}_{it}_{oc}")
                        nc.vector.tensor_copy(ysbh[:], yp[:])
                        nc.sync.dma_start(
                            y[ic * 512 + it * P: ic * 512 + (it + 1) * P,
                              oc * 512:(oc + 1) * 512], ysbh[:])
                return f

            def ojp(ic, it):
                def f():
                    yps_pre[it] = emit_outproj_pre(ic, it, osbs[(ic, 0)])
                return f

            def seq2(f1, f2):
                return lambda: (f1(), f2())

            def aqk(sc, ft, part=None):
                if part is None:
                    return lambda: emit_aqk(sc, ft)
                return lambda: emit_aqk_part(sc, ft, part)

            def av(*sts):
                return lambda: [emit_av(st) for st in sts]

            hooks = {
                (0, 0): av(0, 1), (0, 1): av(2, 3), (0, 2): aqk(1, 2),
                (0, 3): av(4, 5), (0, 4): av(6, 7), (0, 5): aqk(2, 2),
                (0, 6): av(8, 9), (0, 7): av(10, 11), (0, 8): aqk(3, 2),
                (0, 9): av(12, 13), (0, 10): av(14, 15), (0, 11): aqk(0, 1),
                (0, 12): aqk(0, 3), (0, 14): aqk(1, 3),
                (1, 2): aqk(2, 3, 0), (1, 3): aqk(2, 3, 1),
                (1, 4): aqk(2, 3, 2), (1, 5): aqk(2, 3, 3),
                (1, 6): aqk(3, 3, 0), (1, 7): aqk(3, 3, 1),
                (1, 8): aqk(3, 3, 2), (1, 9): aqk(3, 3, 3),
                (1, 10): aqk(1, 0, 0), (1, 11): aqk(1, 0, 1),
                (1, 12): aqk(1, 0, 2), (1, 13): aqk(1, 0, 3),
                (2, 2): ojh(0, 0, 0), (2, 3): ojh(0, 0, 1),
                (2, 4): ojh(0, 1, 0), (2, 5): ojh(0, 1, 1),
                (2, 6): ojh(0, 2, 0), (2, 7): ojh(0, 2, 1),
                (2, 14): ojh(0, 3, 0), (2, 15): ojh(0, 3, 1),
                (3, 2): ojh(0, 4, 0), (3, 3): ojh(0, 4, 1),
                (3, 4): ojh(0, 5, 0), (3, 5): ojh(0, 5, 1),
                (3, 6): ojh(0, 6, 0), (3, 7): ojh(0, 6, 1),
                (3, 12): ojh(0, 7, 0), (3, 13): ojh(0, 7, 1),
                (2, 10): aqk(1, 1, 0), (2, 11): aqk(1, 1, 1),
                (2, 12): aqk(1, 1, 2), (2, 13): aqk(1, 1, 3),
                (3, 8): aqk(2, 0, 0), (3, 9): aqk(2, 0, 1),
                (3, 10): aqk(2, 0, 2), (3, 11): aqk(2, 0, 3),
                (4, 2): ojh(1, 0, 0), (4, 3): ojh(1, 0, 1),
                (4, 4): ojh(1, 1, 0), (4, 5): ojh(1, 1, 1),
                (4, 6): ojh(1, 2, 0), (4, 7): ojh(1, 2, 1),
                (4, 14): ojh(1, 3, 0), (4, 15): ojh(1, 3, 1),
                (5, 2): ojh(1, 4, 0), (5, 3): ojh(1, 4, 1),
                (5, 4): ojh(1, 5, 0), (5, 5): ojh(1, 5, 1),
                (5, 6): ojh(1, 6, 0), (5, 7): ojh(1, 6, 1),
                (5, 12): ojh(1, 7, 0), (5, 13): ojh(1, 7, 1),
                (4, 10): aqk(2, 1, 0), (4, 11): aqk(2, 1, 1),
                (4, 12): aqk(2, 1, 2), (4, 13): aqk(2, 1, 3),
                (5, 8): aqk(3, 0, 0), (5, 9): aqk(3, 0, 1),
                (5, 10): aqk(3, 0, 2), (5, 11): aqk(3, 0, 3),
                (6, 2): ojh(2, 0, 0), (6, 3): ojh(2, 0, 1),
                (6, 4): ojh(2, 1, 0), (6, 5): ojh(2, 1, 1),
                (6, 6): ojh(2, 2, 0), (6, 7): ojh(2, 2, 1),
                (6, 14): ojh(2, 3, 0), (6, 15): ojh(2, 3, 1),
                (7, 2): ojh(2, 4, 0), (7, 3): ojh(2, 4, 1),
                (7, 4): ojh(2, 5, 0), (7, 5): ojh(2, 5, 1),
                (7, 6): ojh(2, 6, 0), (7, 7): ojh(2, 6, 1),
                (7, 8): seq2(ojh(2, 7, 0), ojh(2, 7, 1)),
                (6, 10): aqk(3, 1, 0), (6, 11): aqk(3, 1, 1),
                (6, 12): aqk(3, 1, 2), (6, 13): aqk(3, 1, 3),
                (7, 9): ojp(3, 0), (7, 11): ojp(3, 1),
                (7, 13): ojp(3, 2), (7, 15): ojp(3, 3),
            }

            # ---- main schedule ----
            phases = [(ic, pair) for ic in range(4) for pair in range(2)]

            NWARM = 14
            for w in range(NWARM):
                wacc = ps.tile([P, 512], F32, tag=btag(), name=f"warm_{w}")
                nc.tensor.matmul(wacc[:], wsrc[:, 0:128], wsrc[:],
                                 start=True, stop=True)

            emit_aqk_halves(0, 0)
            emit_aqk_halves(0, 2)
            s_cur = emit_scores(0, 0, 0)
            vse_cur = vse_nxt = None
            for pi, (ic, pair) in enumerate(phases):
                sset = pi % 2
                st8["other"] = 1 - sset
                pvl = [ps.tile([P, 512], F32, tag=f"pvl{sset}{hh}",
                               name=f"pvl_{pi}_{hh}") for hh in range(2)]
                vs_cur = None
                for jt in range(16):
                    if jt == 15:
                        nxt = ((phases[pi + 1][0], phases[pi + 1][1], 0)
                               if pi < 7 else None)
                    else:
                        nxt = (ic, pair, jt + 1)
                    if jt == 15:
                        # rank-1 first so PV(15) carries the stop and
                        # normalization starts immediately
                        emit_void_pvl(pair, pvl, vse_cur, stop=False)
                    s_cur = emit_exp_pvl(ic, pair, jt, s_cur, pvl, nxt,
                                         hooks.get((pi, jt)),
                                         pvstop=True)
                    if pi == 0:
                        if jt == 1:
                            vs_cur = emit_void_scores(ic, pair)
                        elif jt == 3:
                            vse_cur = emit_void_exp(vs_cur, ic, pair)
                    if jt == 0 and pi in (2, 4, 6):
                        # PE-bound phase: void exp after jt0 so PV(0) isn't
                        # delayed behind it at the boundary
                        vse_cur = emit_void_exp(vs_nxt_pend, ic, pair)
                    if jt == 13 and pi < 7:
                        vs_nxt = emit_void_scores(*phases[pi + 1])
                vs_nxt_pend = None
                if pi < 7:
                    if pi + 1 in (2, 4, 6):
                        vs_nxt_pend = vs_nxt      # exp deferred into pi+1
                        vse_nxt = None
                    else:
                        # ACT-bound next phase: void exp fills the boundary
                        # ACT bubble
                        vse_nxt = emit_void_exp(vs_nxt, *phases[pi + 1])
                if pi == 7:
                    last_pvl = pvl
                else:
                    osbs[(ic, pair)] = emit_norm(ic, pair, pvl)
                vse_cur = vse_nxt
            # tail: interleave last-phase norm chunks with the outproj fins
            osb1 = wp.tile([P, 512], BF16, tag="osb1", bufs=2, name="osb_3_1")
            for it in range(4):
                emit_norm_chunk(3, 1, last_pvl, osb1, it)
                emit_outproj_fin(3, it, osb1, yps_pre[it])

    nc.compile()
    return nc


def _voidvo(vv4):
    """[v_h | ones] rank-1 lhsT rows for the void value: [hh, pair, 128]."""
    import ml_dtypes
    out = np.ones((2, 2, P), np.float32)
    for pair in range(2):
        for hh in range(2):
            out[hh, pair, 0:D] = vv4[2 * pair + hh]
    return out.astype(ml_dtypes.bfloat16)


def _prep_inputs(x, w_qkv, w_out, b_out, void_q, void_k, void_v,
                 attention_trace, temperature_factor):
    """Host-side sharding / layout prep. Returns in_maps for 8 cores."""
    import ml_dtypes
    BF = ml_dtypes.bfloat16

    temp = np.maximum(1.0 + np.abs(attention_trace) * temperature_factor,
                      1.0).reshape(HEADS).astype(np.float32)
    scale = (DIM ** -0.5) / temp                       # [16] per head
    qcol_scale = np.repeat(scale, D)                   # [1024]
    wq_scaled = (w_qkv[:, 0:DIM] * qcol_scale[None, :]).astype(np.float32)
    wk = w_qkv[:, DIM:2 * DIM]
    wv_full = w_qkv[:, 2 * DIM:3 * DIM]
    vk = void_k.reshape(HEADS, D)
    vv = void_v.reshape(HEADS, D)

    in_maps = []
    for core in range(8):
        b, hg = divmod(core, 4)
        h0 = hg * HPC
        cs = slice(h0 * D, (h0 + HPC) * D)             # 256 feature cols
        in_maps.append({
            "xT": np.ascontiguousarray(x[b].T).astype(BF),
            "wqkv": np.ascontiguousarray(
                np.concatenate([wq_scaled[:, cs], wk[:, cs],
                                wv_full[:, cs]], axis=1)).astype(BF),
            "wout": np.ascontiguousarray(w_out[cs, :]).astype(BF),
            "voidk": np.ascontiguousarray(vk[h0:h0 + HPC].reshape(2, P)),
            "voidvo": _voidvo(vv[h0:h0 + HPC]),
            "ident_in": np.eye(P, dtype=np.float32),
        })
    return in_maps


def _run(in_maps, trace=False):
    from concourse import bass_utils
    if "nc" not in _cache:
        _cache["nc"] = _build()
    return bass_utils.run_bass_kernel_spmd(
        _cache["nc"], in_maps, core_ids=list(range(8)), trace=trace)


def kernel(x, w_qkv, w_out, b_out, void_q, void_k, void_v,
           attention_trace, temperature_factor):
    args = [np.asarray(a, dtype=np.float32) for a in
            (x, w_qkv, w_out, b_out, void_q, void_k, void_v,
             attention_trace, temperature_factor)]
    in_maps = _prep_inputs(*args)
    res = _run(in_maps)
    out = np.zeros((B, N, DIM), np.float32)
    for core in range(8):
        b = core // 4
        out[b] += np.asarray(res.results[core]["y"], dtype=np.float32)
    out += args[3][None, None, :]                      # b_out
    return out
